# revision 1
# baseline (speedup 1.0000x reference)
"""CapsuleLayer (dynamic routing) Trainium2 kernel.

Self-contained: shards the full inputs over 8 NeuronCores (data-parallel over
batch), runs a Bass/Tile kernel per core, gathers the full output.

Shapes (full): u [256, 1152, 8] f32, W [1152, 10, 16, 8] f32 -> v [256, 10, 16].
Per core: B=32 batches, W replicated.

Math (per core, ROUTING_ITERS=3):
  u_hat[b,i,od] = sum_k W[i,od,k] * u[b,i,k]          (od = o*16+d)
  b0 = 0; for t in 0..2: c = softmax(b, o); s = sum_i c*u_hat; v = squash(s);
  if t<2: b += sum_d u_hat*v

Device layouts (i = jj*16+g, jj<72, g<16; partitions in [.]):
  Wr  [(g,k)=128, (jj,od)=11520]   (host-pretransposed W)
  uT  [(g,k)=128, (jj,b)=2304]     (host-pretransposed u shard)
  BDu [(g,k)=128, (jj,b8,g')]      block-diag u, host-packed, DMA-streamed
  u_hat [(b8,g16)=128, (jj,od)]    built by PE: BDu.T @ Wr  (per 8-batch block)
  s matmul: lhsT = block-diag c [(b8,g16),(b8',o)], rhs = u_hat -> psum[(b',o),od]
"""

import os
import sys

import numpy as np

for _p in ("/opt/trn_rl_repo", "/root/.axon_site/_ro/trn_rl_repo"):
    if os.path.isdir(_p) and _p not in sys.path:
        sys.path.insert(0, _p)

import concourse.bacc as bacc
import concourse.bass as bass
import concourse.mybir as mybir
import concourse.tile as tile

F32 = mybir.dt.float32


def _register_scan_mac():
    """Custom DVE op: out[p,k] = cumsum_k(in0*in1) (fp32 state).

    Used for the agreement step: running sum of u_hat*v, with per-(jj,o)
    segment sums recovered from differences at 16-element boundaries.
    """
    import numpy as np

    from concourse import dve_ops as dops
    from concourse.dve_spec import AluOp, Spec, Src0, Src1, lower, scan
    from concourse.dve_uop import DveOpSpec

    name = "SCAN_MAC_ANT"
    if any(op.name == name for op in dops.OPS):
        return name
    spec = Spec(
        body=scan(AluOp.ADD, Src0 * Src1),
        reference=lambda in0, in1, c0, c1, c2: np.cumsum(
            np.asarray(in0, np.float32).reshape(in0.shape[0], -1)
            * np.asarray(in1, np.float32).reshape(in1.shape[0], -1),
            axis=-1,
        ).reshape(in0.shape),
    )
    shas = {}
    for ver in ("v3", "v4"):
        uops = lower(spec, ver=ver)
        shas[ver] = DveOpSpec(
            name=name, opcode=0, uops=uops, rd1_en=True
        ).sha(ver)
    op = dops.DveOp(name, spec, subdim=False, uops_sha=shas)
    dops.OPS.append(op)
    dops.CUSTOM_DVE_SPECS[name] = spec
    dops._SUB_OPCODE_FOR_NAME[name] = dops._CUSTOM_DVE_ROW_BASE + len(dops.OPS) - 1
    assert dops._SUB_OPCODE_FOR_NAME[name] < 0x20
    return op


_SCAN_MAC = _register_scan_mac()

# Problem constants (per core)
B = 32          # local batch (256 / 8 cores)
I = 1152        # in capsules
O = 10          # out capsules
D = 16          # out dim
K = 8           # in dim
JJ = 72         # i groups of 16
G = 16          # group size
OD = O * D      # 160
BB = 8          # batch block (psum/output partition packing)
NBLK = B // BB  # 4
N_ITERS = 3


def _ap(base, free_dims, extra_offset=0):
    """AP with the base's partition dim and explicit free [step, count] dims."""
    return bass.AP(
        tensor=base.tensor,
        offset=base.offset + extra_offset,
        ap=[list(base.ap[0])] + [list(d) for d in free_dims],
    )


def _squash(nc, pool, s_sb, p, v_out):
    """squash over d (16) per o segment. s_sb: [p, 160] f32 sbuf -> v_out."""
    sq = pool.tile([p, OD], F32, tag="sq")
    nc.scalar.square(sq, s_sb)
    nsq = pool.tile([p, O], F32, tag="nsq")
    nc.vector.reduce_sum(
        out=nsq, in_=sq[:].rearrange("p (o d) -> p o d", d=D),
        axis=mybir.AxisListType.X,
    )
    # sqrt(x) = exp(0.5*ln(x)) — keeps ACT on one table set (ln/exp)
    rt = pool.tile([p, O], F32, tag="rt")
    nc.scalar.activation(rt, nsq, mybir.ActivationFunctionType.Ln)
    nc.scalar.activation(rt, rt, mybir.ActivationFunctionType.Exp, scale=0.5)
    nc.vector.tensor_scalar_add(rt, rt, 1e-8)     # + eps
    op1 = pool.tile([p, O], F32, tag="op1")
    nc.vector.tensor_scalar_add(op1, nsq, 1.0)    # 1 + |s|^2
    nc.vector.tensor_mul(op1, op1, rt)            # (1+n)(sqrt+eps)
    rec = pool.tile([p, O], F32, tag="rec")
    nc.vector.reciprocal(rec, op1)
    nc.vector.tensor_mul(rec, rec, nsq)           # n/((1+n)(sqrt+eps))
    nc.vector.tensor_mul(
        v_out[:].rearrange("p (o d) -> p o d", d=D),
        s_sb[:].rearrange("p (o d) -> p o d", d=D),
        _ap(rec[:], [[1, O], [0, D]]),
    )
    return v_out


def _pin_act_table():
    """Make every ACT function we use resolve to the one set containing all
    of them (natural_log_exp_and_others), so bacc hoists a single
    InstLoadActFuncSet instead of thrashing Exp<->Ln sets (~1.3us/load)."""
    from concourse.bacc import get_activation_tables

    tabs = get_activation_tables("gen3")
    keep = "natural_log_exp_and_others"
    if keep not in tabs:
        return
    ours = {
        mybir.ActivationFunctionType.Exp,
        mybir.ActivationFunctionType.Ln,
        mybir.ActivationFunctionType.Square,
        mybir.ActivationFunctionType.Copy,
        mybir.ActivationFunctionType.Identity,
    }
    if not ours <= tabs[keep]:
        return
    for name, s in tabs.items():
        if name != keep:
            s -= ours


def build_program():
    _pin_act_table()
    nc = bacc.Bacc("TRN2")
    wr_d = nc.dram_tensor("wr", [128, JJ * OD], F32, kind="ExternalInput")
    ut_d = nc.dram_tensor("ut", [128, JJ * B], F32, kind="ExternalInput")
    # block-diag u, host-packed contiguous per (blk, ch): [4, 8, 128, 1152]
    bdu_d = nc.dram_tensor(
        "bdu", [NBLK * 8 * 128, 9 * BB * G], F32, kind="ExternalInput"
    )
    mb_d = nc.dram_tensor("maskb", [128, BB * O], F32, kind="ExternalInput")
    md_d = nc.dram_tensor("maskd", [128, OD], F32, kind="ExternalInput")
    out_d = nc.dram_tensor("v_out", [B, OD], F32, kind="ExternalOutput")

    with tile.TileContext(nc) as tc:
        with (
            tc.tile_pool(name="persist", bufs=1) as persist,
            tc.tile_pool(name="uhat", bufs=2) as uhat_pool,
            tc.tile_pool(name="bdu", bufs=2) as bdu_pool,
            tc.tile_pool(name="ascr", bufs=2) as ascr_pool,
            tc.tile_pool(name="cbd", bufs=2) as cbd_pool,
            tc.tile_pool(name="blog", bufs=2) as blog_pool,
            tc.tile_pool(name="cbuf", bufs=2) as cbuf_pool,
            tc.tile_pool(name="small", bufs=2) as small,
            tc.tile_pool(name="pb", bufs=4, space="PSUM") as pb_pool,
            tc.tile_pool(name="ps", bufs=2, space="PSUM") as ps_pool,
            tc.tile_pool(name="ps0", bufs=1, space="PSUM") as ps0_pool,
        ):
            # ---- resident loads ----
            wr = persist.tile([128, JJ, OD], F32)
            for ch in range(8):
                nc.sync.dma_start(
                    out=wr[:, ch * 9 : (ch + 1) * 9, :],
                    in_=wr_d[:, ch * 9 * OD : (ch + 1) * 9 * OD].rearrange(
                        "p (a b) -> p a b", b=OD
                    ),
                )
            ut = persist.tile([128, JJ, B], F32)
            nc.sync.dma_start(
                out=ut, in_=ut_d[:].rearrange("p (a b) -> p a b", b=B)
            )
            maskb = persist.tile([128, BB * O], F32)
            nc.sync.dma_start(out=maskb, in_=mb_d[:])
            maskd = persist.tile([128, OD], F32)
            nc.sync.dma_start(out=maskd, in_=md_d[:])

            # ---- s0 = 0.1 * sum_i u_hat  (dense (i,k) contraction) ----
            s0_ps = ps0_pool.tile([B, OD], F32)
            for jj in range(JJ):
                nc.tensor.matmul(
                    s0_ps, lhsT=ut[:, jj, :], rhs=wr[:, jj, :],
                    start=(jj == 0), stop=(jj == JJ - 1),
                )
            s0_sb = small.tile([B, OD], F32, tag="s0")
            nc.scalar.activation(
                s0_sb, s0_ps, mybir.ActivationFunctionType.Copy, scale=0.1
            )
            v0 = persist.tile([B, OD], F32, tag="v0")
            _squash(nc, small, s0_sb, B, v0)  # [32, 160]

            # uniform-c lhsT for t=0 (shared by all blks/jj)
            cbd0 = persist.tile([128, BB * O], F32, tag="cbd0")
            nc.scalar.mul(cbd0, maskb, 0.1)

            # ---- per 8-batch block: build u_hat then route ----
            for blk in range(NBLK):
                u_hat = uhat_pool.tile([128, JJ, OD], F32)
                for ch in range(8):  # 9 jj per chunk
                    bdu = bdu_pool.tile([128, 9, BB, G], F32)
                    nc.sync.dma_start(
                        out=bdu,
                        in_=bdu_d[
                            (blk * 8 + ch) * 128 : (blk * 8 + ch + 1) * 128, :
                        ].rearrange("p (a b g) -> p a b g", b=BB, g=G),
                    )
                    for j3 in range(3):  # 3-jj groups share one psum bank
                        ps = pb_pool.tile([128, 3, OD], F32)
                        for j in range(3):
                            jj = ch * 9 + j3 * 3 + j
                            nc.tensor.matmul(
                                ps[:, j, :], lhsT=bdu[:, j3 * 3 + j, :, :],
                                rhs=wr[:, jj, :], start=True, stop=True,
                            )
                        jj0 = ch * 9 + j3 * 3
                        nc.scalar.copy(u_hat[:, jj0 : jj0 + 3, :], ps)

                blog = blog_pool.tile([128, JJ, O], F32)
                vcur = None  # [BB or B, 160] sbuf tile holding v_t rows for blk
                for t in range(N_ITERS):
                    # -- agreement (t>0 uses previous v) and logits update --
                    if t == 0:
                        pass  # b=0 -> c uniform handled via s0 path
                    else:
                        vrep = small.tile([128, OD], F32, tag="vrep")
                        vr = vrep[:]
                        vr_ps = vr.ap[0][0]
                        if t == 1:
                            src = _ap(
                                v0[:], [[0, G], [1, OD]],
                                extra_offset=0,
                            )
                            # restrict partition dim to this block's 8 rows
                            src = bass.AP(
                                tensor=src.tensor,
                                offset=src.offset
                                + blk * BB * v0[:].ap[0][0],
                                ap=[[v0[:].ap[0][0], BB]] + list(src.ap)[1:],
                            )
                        else:
                            assert vcur is not None
                            vc = vcur[:]
                            vps = vc.ap[0][0]
                            vtmp = small.tile([BB, OD], F32, tag="vtmp")
                            nc.sync.dma_start(out=vtmp, in_=vcur)
                            vt = vtmp[:]
                            src = bass.AP(
                                tensor=vt.tensor, offset=vt.offset,
                                ap=[[vt.ap[0][0], BB], [0, G], [1, OD]],
                            )
                        nc.sync.dma_start(out=vrep, in_=src)
                        # fused scan-MAC: S = cumsum(u_hat * v) per chunk;
                        # per-(jj,o) sums = S[16n+15] - S[16n-1]
                        AC = 9  # jj per agreement chunk
                        NSEG = AC * O  # segments per chunk
                        for h in range(JJ // AC):
                            scr = ascr_pool.tile([128, AC * OD], F32)
                            nc.vector._custom_dve(
                                _SCAN_MAC,
                                out=scr,
                                in0=u_hat[:, h * AC : (h + 1) * AC, :],
                                in1=_ap(vrep[:], [[0, AC], [1, OD]]),
                            )
                            sv = scr[:]
                            s_hi = bass.AP(
                                tensor=sv.tensor, offset=sv.offset + D - 1,
                                ap=[list(sv.ap[0]), [D, NSEG]],
                            )
                            s_lo = bass.AP(
                                tensor=sv.tensor, offset=sv.offset + D - 1,
                                ap=[list(sv.ap[0]), [D, NSEG - 1]],
                            )
                            bl = blog[:, h * AC : (h + 1) * AC, :]
                            bl_flat = bl.rearrange("p a o -> p (a o)")
                            if t == 1:
                                nc.vector.tensor_copy(bl_flat, s_hi)
                            else:
                                nc.vector.tensor_add(bl_flat, bl_flat, s_hi)
                            nc.vector.tensor_sub(
                                bl_flat[:, 1:NSEG],
                                bl_flat[:, 1:NSEG],
                                s_lo,
                            )

                    # -- c = softmax(blog) over o; then s matmul --
                    if t == 0:
                        # uniform c: lhsT = 0.1 * maskb, same for every jj
                        pass
                    else:
                        # logits are bounded (||v||<1 => |logit| <~ 16),
                        # so exp without max-subtraction is fp32-safe
                        cb = cbuf_pool.tile([128, JJ, O], F32)
                        nc.scalar.activation(
                            cb, blog, mybir.ActivationFunctionType.Exp
                        )
                        ssum = small.tile([128, JJ], F32, tag="ssum")
                        nc.vector.reduce_sum(
                            out=ssum, in_=cb, axis=mybir.AxisListType.X
                        )
                        rec = small.tile([128, JJ], F32, tag="srec")
                        nc.vector.reciprocal(rec, ssum)
                        nc.gpsimd.tensor_mul(
                            cb, cb, _ap(rec[:], [[1, JJ], [0, O]])
                        )

                    s_ps = ps_pool.tile([BB * O, OD], F32)
                    if t == 0:
                        for jj in range(JJ):
                            nc.tensor.matmul(
                                s_ps, lhsT=cbd0, rhs=u_hat[:, jj, :],
                                start=(jj == 0), stop=(jj == JJ - 1),
                            )
                    else:
                        for ch4 in range(8):  # 9-jj cbd chunks
                            cbd = cbd_pool.tile([128, 9, BB, O], F32)
                            ceng = nc.gpsimd if ch4 % 4 != 3 else nc.vector
                            ceng.tensor_mul(
                                cbd,
                                _ap(cb[:], [[O, 9], [0, BB], [1, O]],
                                    extra_offset=ch4 * 9 * O),
                                _ap(maskb[:], [[0, 9], [O, BB], [1, O]]),
                            )
                            for j in range(9):
                                jj = ch4 * 9 + j
                                nc.tensor.matmul(
                                    s_ps, lhsT=cbd[:, j, :, :],
                                    rhs=u_hat[:, jj, :],
                                    start=(jj == 0), stop=(jj == JJ - 1),
                                )

                    # -- diag extract: s80[(b,o), d] = s_ps[(b,o), o*16+d]
                    #    via constant diag mask + reduce over o' --
                    sdm = small.tile([O * BB, OD], F32, tag="sdm")
                    nc.vector.tensor_mul(sdm, s_ps, maskd[: O * BB, :])
                    s80 = small.tile([O * BB, D], F32, tag="s80")
                    nc.vector.reduce_sum(
                        out=s80,
                        in_=sdm[:].rearrange("p (o d) -> p d o", d=D),
                        axis=mybir.AxisListType.X,
                    )
                    # squash on [(o,b), d] with per-partition scalars
                    nsq = small.tile([O * BB, 1], F32, tag="nsq80")
                    sq = small.tile([O * BB, D], F32, tag="sq80")
                    nc.scalar.square(sq, s80)
                    nc.vector.reduce_sum(
                        out=nsq, in_=sq, axis=mybir.AxisListType.X
                    )
                    # squash factor ~= sqrt(nsq)/(1+nsq)  (eps negligible);
                    # sqrt via exp(0.5*ln) to stay on one ACT table set
                    rt = small.tile([O * BB, 1], F32, tag="rt80")
                    nc.scalar.activation(
                        rt, nsq, mybir.ActivationFunctionType.Ln
                    )
                    nc.scalar.activation(
                        rt, rt, mybir.ActivationFunctionType.Exp, scale=0.5
                    )
                    op1 = small.tile([O * BB, 1], F32, tag="op180")
                    nc.vector.tensor_scalar_add(op1, nsq, 1.0)
                    rec = small.tile([O * BB, 1], F32, tag="rec80")
                    nc.vector.reciprocal(rec, op1)
                    nc.vector.tensor_mul(rec, rec, rt)
                    vcur = small.tile([O * BB, D], F32, tag="vcur")
                    nc.vector.tensor_scalar_mul(vcur, s80, rec)

                # v_out[blk*8+b, o*16+d] = vcur[b*10+o, d] (same flat order)
                nc.sync.dma_start(
                    out=out_d[blk * BB : (blk + 1) * BB, :], in_=vcur
                )
    nc.compile()
    return nc


# ---------------- host side ----------------

_NC_CACHE = None


def _get_nc():
    global _NC_CACHE
    if _NC_CACHE is None:
        _NC_CACHE = build_program()
    return _NC_CACHE


def _pack_wr(W):
    # Wr[g*8+k, jj*160 + o*16 + d] = W[jj*16+g, o, d, k]
    return np.ascontiguousarray(
        W.reshape(JJ, G, O, D, K).transpose(1, 4, 0, 2, 3).reshape(128, JJ * OD)
    ).astype(np.float32)


def _pack_ut(u_loc):
    # uT[g*8+k, jj*B + b] = u_loc[b, jj*16+g, k]
    return np.ascontiguousarray(
        u_loc.reshape(B, JJ, G, K).transpose(2, 3, 1, 0).reshape(128, JJ * B)
    ).astype(np.float32)


def _masks():
    p = np.arange(128)
    mb = (np.arange(BB)[None, :] == (p // G)[:, None]).astype(np.float32)
    mb = np.repeat(mb, O, axis=1)  # [128, 80] over (b', o)
    # maskd[(b,o) p<80, o'*16+d] = (o' == o); rows >=80 zero
    md = np.zeros((128, OD), dtype=np.float32)
    po = np.arange(O * BB) % O
    for od in range(OD):
        md[: O * BB, od] = (od // D == po).astype(np.float32)
    return mb, md


def _pack_bdu(u_loc):
    # bdu[(blk,ch)*128 + g*8+k, (j, b, g')] = u_loc[blk*8+b, (ch*9+j)*16+g', k]
    #   nonzero only when g' == g; contiguous per (blk, ch) slice.
    u4 = u_loc.reshape(NBLK, BB, JJ // 9, 9, G, K)  # (blk, b, ch, j, g, k)
    out = np.zeros((NBLK, 8, G, K, 9, BB, G), dtype=np.float32)
    for g in range(G):
        # (blk, ch, k, j, b)
        out[:, :, g, :, :, :, g] = u4[:, :, :, :, g, :].transpose(0, 2, 4, 3, 1)
    return np.ascontiguousarray(out.reshape(NBLK * 8 * 128, 9 * BB * G))


LAST_RESULTS = None


def kernel(u, W):
    from concourse.bass_utils import run_bass_kernel_spmd

    global LAST_RESULTS
    u = np.asarray(u, dtype=np.float32)
    W = np.asarray(W, dtype=np.float32)
    nc = _get_nc()
    wr = _pack_wr(W)
    mb, md = _masks()
    in_maps = []
    for c in range(8):
        u_loc = u[c * B : (c + 1) * B]
        in_maps.append(
            {
                "wr": wr,
                "ut": _pack_ut(u_loc),
                "bdu": _pack_bdu(u_loc),
                "maskb": mb,
                "maskd": md,
            }
        )
    trace = bool(int(os.environ.get("KBENCH_TRACE", "0")))
    try:
        res = run_bass_kernel_spmd(
            nc, in_maps, core_ids=list(range(8)), trace=trace
        )
    except ModuleNotFoundError:
        # axon NTFF hook unavailable in this container; run without trace
        res = run_bass_kernel_spmd(nc, in_maps, core_ids=list(range(8)))
    LAST_RESULTS = res
    outs = [r["v_out"].reshape(B, O, D) for r in res.results]
    return np.concatenate(outs, axis=0).astype(np.float32)



# revision 16
# speedup vs baseline: 1.9877x; 1.9877x over previous
"""CapsuleLayer (dynamic routing) Trainium2 kernel.

Self-contained: shards the full inputs over 8 NeuronCores (data-parallel over
batch), runs a Bass/Tile kernel per core, gathers the full output.

Shapes (full): u [256, 1152, 8] f32, W [1152, 10, 16, 8] f32 -> v [256, 10, 16].
Per core: B=32 batches, W replicated.

Math (per core, ROUTING_ITERS=3):
  u_hat[b,i,od] = sum_k W[i,od,k] * u[b,i,k]          (od = o*16+d)
  b0 = 0; for t in 0..2: c = softmax(b, o); s = sum_i c*u_hat; v = squash(s);
  if t<2: b += sum_d u_hat*v

Device layouts (i = jj*16+g, jj<72, g<16; partitions in [.]):
  Wr  [(g,k)=128, (jj,od)=11520]   f16 (host-pretransposed W)
  uT  [(g,k)=128, (jj,b)=2304]     f16 (host-pretransposed u shard)
  BDu [(g,k)=128, (jj,b8,g')]      f16 block-diag u, host-packed, DMA-streamed
  u_hat [(b8,g16)=128, (jj,od)]    f16, built by PE: BDu.T @ Wr per 8-batch blk
  s matmul: lhsT = block-diag c (f16, built by 4x-mode partition-slice copies
  into a memset-once tile), rhs = u_hat -> psum[(b',o),od] f32
  t=0 needs no per-blk s pass: v0 comes from the dense ut@wr contraction.
  Agreement sum_d u_hat*v: jj 0..49 on DVE (fused scan-MAC + strided diff),
  jj 50..71 on Pool (mul + 4-level tree reduce). Blocks are software-pipelined:
  stage s emits build(s), route(s-2, t2), route(s-1, t1).
"""

import os
import sys

import numpy as np

for _p in ("/opt/trn_rl_repo", "/root/.axon_site/_ro/trn_rl_repo"):
    if os.path.isdir(_p) and _p not in sys.path:
        sys.path.insert(0, _p)

import concourse.bacc as bacc
import concourse.bass as bass
import concourse.mybir as mybir
import concourse.tile as tile

F32 = mybir.dt.float32
F16 = mybir.dt.float16


def _register_scan_mac():
    """Custom DVE op: out[p,k] = cumsum_k(in0*in1) (fp32 state)."""
    import numpy as np

    from concourse import dve_ops as dops
    from concourse.dve_spec import AluOp, Spec, Src0, Src1, lower, scan
    from concourse.dve_uop import DveOpSpec

    name = "SCAN_MAC_ANT"
    for op in dops.OPS:
        if op.name == name:
            return op
    spec = Spec(
        body=scan(AluOp.ADD, Src0 * Src1),
        reference=lambda in0, in1, c0, c1, c2: np.cumsum(
            np.asarray(in0, np.float32).reshape(in0.shape[0], -1)
            * np.asarray(in1, np.float32).reshape(in1.shape[0], -1),
            axis=-1,
        ).reshape(in0.shape),
    )
    shas = {}
    for ver in ("v3", "v4"):
        uops = lower(spec, ver=ver)
        shas[ver] = DveOpSpec(
            name=name, opcode=0, uops=uops, rd1_en=True
        ).sha(ver)
    op = dops.DveOp(name, spec, subdim=False, uops_sha=shas)
    dops.OPS.append(op)
    dops.CUSTOM_DVE_SPECS[name] = spec
    dops._SUB_OPCODE_FOR_NAME[name] = dops._CUSTOM_DVE_ROW_BASE + len(dops.OPS) - 1
    assert dops._SUB_OPCODE_FOR_NAME[name] < 0x20
    return op


_SCAN_MAC = _register_scan_mac()

# Problem constants (per core)
B = 32          # local batch (256 / 8 cores)
I = 1152        # in capsules
O = 10          # out capsules
D = 16          # out dim
K = 8           # in dim
JJ = 72         # i groups of 16
G = 16          # group size
OD = O * D      # 160
BB = 8          # batch block (psum/output partition packing)
NBLK = B // BB  # 4
N_ITERS = 3
AC = 26         # jj per DVE agreement scan chunk
NDVE = 2        # scan chunks per (blk,t) on DVE
PJJ = JJ - NDVE * AC  # jj handled by Pool (mul + tree reduce)


def _ap(base, free_dims, extra_offset=0):
    """AP with the base's partition dim and explicit free [step, count] dims."""
    return bass.AP(
        tensor=base.tensor,
        offset=base.offset + extra_offset,
        ap=[list(base.ap[0])] + [list(d) for d in free_dims],
    )


def _squash(nc, pool, s_sb, p, v_out):
    """squash over d (16) per o segment. s_sb: [p, 160] f32 sbuf -> v_out."""
    sq = pool.tile([p, OD], F32, tag="sq")
    nc.scalar.square(sq, s_sb)
    nsq = pool.tile([p, O], F32, tag="nsq")
    nc.vector.reduce_sum(
        out=nsq, in_=sq[:].rearrange("p (o d) -> p o d", d=D),
        axis=mybir.AxisListType.X,
    )
    # sqrt(x) = exp(0.5*ln(x)) — keeps ACT on one table set (ln/exp)
    rt = pool.tile([p, O], F32, tag="rt")
    nc.scalar.activation(rt, nsq, mybir.ActivationFunctionType.Ln)
    nc.scalar.activation(rt, rt, mybir.ActivationFunctionType.Exp, scale=0.5)
    nc.vector.tensor_scalar_add(rt, rt, 1e-8)     # + eps
    op1 = pool.tile([p, O], F32, tag="op1")
    nc.vector.tensor_scalar_add(op1, nsq, 1.0)    # 1 + |s|^2
    nc.vector.tensor_mul(op1, op1, rt)            # (1+n)(sqrt+eps)
    rec = pool.tile([p, O], F32, tag="rec")
    nc.vector.reciprocal(rec, op1)
    nc.vector.tensor_mul(rec, rec, nsq)           # n/((1+n)(sqrt+eps))
    nc.vector.tensor_mul(
        v_out[:].rearrange("p (o d) -> p o d", d=D),
        s_sb[:].rearrange("p (o d) -> p o d", d=D),
        _ap(rec[:], [[1, O], [0, D]]),
    )
    return v_out


def _pin_act_table():
    """Make every ACT function we use resolve to the one set containing all
    of them (natural_log_exp_and_others), so bacc hoists a single
    InstLoadActFuncSet instead of thrashing Exp<->Ln sets (~1.3us/load)."""
    from concourse.bacc import get_activation_tables

    tabs = get_activation_tables("gen3")
    keep = "natural_log_exp_and_others"
    if keep not in tabs:
        return
    ours = {
        mybir.ActivationFunctionType.Exp,
        mybir.ActivationFunctionType.Ln,
        mybir.ActivationFunctionType.Square,
        mybir.ActivationFunctionType.Copy,
        mybir.ActivationFunctionType.Identity,
    }
    if not ours <= tabs[keep]:
        return
    for name, s in tabs.items():
        if name != keep:
            s -= ours


def build_program():
    _pin_act_table()
    nc = bacc.Bacc("TRN2")
    wr_d = nc.dram_tensor("wr", [128, JJ * OD], F16, kind="ExternalInput")
    ut_d = nc.dram_tensor("ut", [128, JJ * B], F16, kind="ExternalInput")
    # block-diag u, host-packed contiguous per (blk, ch): [4*8*128, 1152]
    bdu_d = nc.dram_tensor(
        "bdu", [NBLK * 8 * 128, 9 * BB * G], F16, kind="ExternalInput"
    )
    mb_d = nc.dram_tensor("maskb", [128, BB * O], F16, kind="ExternalInput")
    md_d = nc.dram_tensor("maskd", [128, OD], F32, kind="ExternalInput")
    out_d = nc.dram_tensor("v_out", [B, OD], F32, kind="ExternalOutput")

    with tile.TileContext(nc) as tc:
        with (
            tc.tile_pool(name="persist", bufs=1) as persist,
            tc.tile_pool(name="uhat", bufs=3) as uhat_pool,
            tc.tile_pool(name="bdu", bufs=2) as bdu_pool,
            tc.tile_pool(name="pp", bufs=2) as pp_pool,
            tc.tile_pool(name="blog", bufs=2) as blog_pool,
            tc.tile_pool(name="cbuf", bufs=2) as cbuf_pool,
            tc.tile_pool(name="small", bufs=3) as small,
            tc.tile_pool(name="pb", bufs=4, space="PSUM") as pb_pool,
            tc.tile_pool(name="ps", bufs=2, space="PSUM") as ps_pool,
            tc.tile_pool(name="ps0", bufs=1, space="PSUM") as ps0_pool,
        ):
            # ---- resident loads (bulk on the Act HWDGE queue) ----
            ut = persist.tile([128, JJ, B], F16)
            nc.scalar.dma_start(
                out=ut, in_=ut_d[:].rearrange("p (a b) -> p a b", b=B)
            )
            wr = persist.tile([128, JJ, OD], F16)

            def load_wr(ch):
                nc.scalar.dma_start(
                    out=wr[:, ch * 9 : (ch + 1) * 9, :],
                    in_=wr_d[:, ch * 9 * OD : (ch + 1) * 9 * OD].rearrange(
                        "p (a b) -> p a b", b=OD
                    ),
                )

            load_wr(0)
            maskb = persist.tile([128, BB * O], F16)
            nc.sync.dma_start(out=maskb, in_=mb_d[:])
            maskd = persist.tile([128, OD], F32)
            nc.sync.dma_start(out=maskd, in_=md_d[:])

            # scan scratch: [128, 1+AC*OD] f32, col 0 pinned to zero so the
            # per-segment sums are single strided subs (S[16n+16]-S[16n]).
            scr = persist.tile([128, 1 + AC * OD], F32)
            nc.vector.memset(scr[:, 0:1], 0.0)
            # block-diag c lhsT tiles, rebuilt by masked multiply per route
            cbd_t = [persist.tile([128, JJ, BB, O], F16, tag=f"cbd{i}",
                                  name=f"cbd{i}")
                     for i in range(2)]

            v0 = persist.tile([B, OD], F32, tag="v0")

            def compute_v0():
                # s0 = 0.1 * sum_i u_hat  (dense (i,k) contraction)
                s0_ps = ps0_pool.tile([B, OD], F32, name="s0_ps")
                for jj in range(JJ):
                    nc.tensor.matmul(
                        s0_ps, lhsT=ut[:, jj, :], rhs=wr[:, jj, :],
                        start=(jj == 0), stop=(jj == JJ - 1),
                    )
                s0_sb = small.tile([B, OD], F32, tag="s0", name="s0_sb")
                nc.scalar.activation(
                    s0_sb, s0_ps, mybir.ActivationFunctionType.Copy, scale=0.1
                )
                _squash(nc, small, s0_sb, B, v0)  # [32, 160]

            state = {}  # blk -> dict(u_hat, blog, blog2, vcur)

            def build(blk):
                u_hat = uhat_pool.tile([128, JJ, OD], F16, name="u_hat")
                state[blk] = {"u_hat": u_hat}
                for ch in range(8):  # 9 jj per chunk
                    bdu = bdu_pool.tile([128, 9, BB, G], F16, name="bdu")
                    nc.scalar.dma_start(
                        out=bdu,
                        in_=bdu_d[
                            (blk * 8 + ch) * 128 : (blk * 8 + ch + 1) * 128, :
                        ].rearrange("p (a b g) -> p a b g", b=BB, g=G),
                    )
                    if blk == 0 and ch < 7:
                        load_wr(ch + 1)
                    for j3 in range(3):  # 3-jj groups share one psum bank
                        ps = pb_pool.tile([128, 3, OD], F32, name="ps")
                        for j in range(3):
                            jj = ch * 9 + j3 * 3 + j
                            nc.tensor.matmul(
                                ps[:, j, :], lhsT=bdu[:, j3 * 3 + j, :, :],
                                rhs=wr[:, jj, :], start=True, stop=True,
                            )
                        jj0 = ch * 9 + j3 * 3
                        nc.scalar.copy(u_hat[:, jj0 : jj0 + 3, :], ps)

            def route_agree(blk, t):
                st = state[blk]
                u_hat = st["u_hat"]
                # -- vrep[(b,g), od] = v_{t-1}[b, od] --
                vrep = small.tile([128, OD], F32, tag="vrep", name="vrep")
                if t == 1:
                    src = _ap(v0[:], [[0, G], [1, OD]])
                    src = bass.AP(
                        tensor=src.tensor,
                        offset=src.offset + blk * BB * v0[:].ap[0][0],
                        ap=[[v0[:].ap[0][0], BB]] + list(src.ap)[1:],
                    )
                else:
                    vtmp = small.tile([BB, OD], F32, tag="vtmp", name="vtmp")
                    nc.sync.dma_start(out=vtmp, in_=st["vcur"])
                    vt = vtmp[:]
                    src = bass.AP(
                        tensor=vt.tensor, offset=vt.offset,
                        ap=[[vt.ap[0][0], BB], [0, G], [1, OD]],
                    )
                nc.sync.dma_start(out=vrep, in_=src)

                # -- agreement a[b,i,o] = sum_d u_hat*v into bl --
                if t == 1:
                    bl = blog_pool.tile([128, JJ, O], F32, tag="blog",
                                        name="blog")
                    st["blog"] = bl
                else:
                    bl = blog_pool.tile([128, JJ, O], F32, tag="blog2",
                                        name="blog2")
                # DVE chunks: fused scan-MAC cumsum, segment sums by
                # strided subtraction against the zero-led scratch.
                for h in range(NDVE):
                    nc.vector._custom_dve(
                        _SCAN_MAC,
                        out=scr[:, 1 : 1 + AC * OD],
                        in0=u_hat[:, h * AC : (h + 1) * AC, :],
                        in1=_ap(vrep[:], [[0, AC], [1, OD]]),
                    )
                    sv = scr[:]
                    nseg = AC * O
                    s_hi = _ap(sv, [[D, nseg]], extra_offset=D)
                    s_lo = _ap(sv, [[D, nseg]], extra_offset=0)
                    blf = bl[:, h * AC : (h + 1) * AC, :].rearrange(
                        "p a o -> p (a o)"
                    )
                    nc.vector.tensor_sub(blf, s_hi, s_lo)
                # Pool chunk: products then 4-level tree reduce over d.
                jp = NDVE * AC
                nsegp = PJJ * O
                pp = pp_pool.tile([128, PJJ * OD], F16, name="pp")
                nc.gpsimd.tensor_mul(
                    pp,
                    u_hat[:, jp:, :].rearrange("p a od -> p (a od)"),
                    _ap(vrep[:], [[0, PJJ], [1, OD]]),
                )
                tr1 = pp_pool.tile([128, nsegp * 8], F16, tag="tr1",
                                   name="tr1")
                nc.gpsimd.tensor_add(
                    tr1[:].rearrange("p (s d) -> p s d", d=8),
                    _ap(pp[:], [[D, nsegp], [1, 8]]),
                    _ap(pp[:], [[D, nsegp], [1, 8]], extra_offset=8),
                )
                tr2 = pp_pool.tile([128, nsegp * 4], F16, tag="tr2",
                                   name="tr2")
                nc.gpsimd.tensor_add(
                    tr2[:].rearrange("p (s d) -> p s d", d=4),
                    _ap(tr1[:], [[8, nsegp], [1, 4]]),
                    _ap(tr1[:], [[8, nsegp], [1, 4]], extra_offset=4),
                )
                tr3 = pp_pool.tile([128, nsegp * 2], F16, tag="tr3",
                                   name="tr3")
                nc.gpsimd.tensor_add(
                    tr3[:].rearrange("p (s d) -> p s d", d=2),
                    _ap(tr2[:], [[4, nsegp], [1, 2]]),
                    _ap(tr2[:], [[4, nsegp], [1, 2]], extra_offset=2),
                )
                blfp = bl[:, jp:, :].rearrange("p a o -> p (a o)")
                nc.gpsimd.tensor_add(
                    blfp,
                    _ap(tr3[:], [[2, nsegp]]),
                    _ap(tr3[:], [[2, nsegp]], extra_offset=1),
                )

                st["bl"] = bl

            def route_tail(blk, t):
                st = state[blk]
                u_hat = st["u_hat"]
                bl = st["bl"]
                # -- c = softmax(blog[+blog2]) over o --
                if t == 1:
                    blogs = bl
                else:
                    blogs = st["blog"]
                    nc.vector.tensor_add(blogs, blogs, bl)
                cb32 = cbuf_pool.tile([128, JJ, O], F32, tag="cb32",
                                      name="cb32")
                nc.scalar.activation(
                    cb32, blogs, mybir.ActivationFunctionType.Exp
                )
                ssum = small.tile([128, JJ], F32, tag="ssum", name="ssum")
                nc.vector.reduce_sum(
                    out=ssum, in_=cb32, axis=mybir.AxisListType.X
                )
                rec = small.tile([128, JJ], F32, tag="srec", name="srec")
                nc.vector.reciprocal(rec, ssum)
                cb2 = cbuf_pool.tile([128, JJ, O], F16, tag="cb2", name="cb2")
                nc.vector.tensor_mul(
                    cb2, cb32, _ap(rec[:], [[1, JJ], [0, O]])
                )

                # -- cbd = cb2 (bcast over b') * maskb (bcast over jj) --
                # all-f16 packed SBUF operands -> DVE 2x mode
                cbd = cbd_t[(blk * 2 + t) % 2]
                nc.vector.tensor_mul(
                    cbd,
                    _ap(cb2[:], [[O, JJ], [0, BB], [1, O]]),
                    _ap(maskb[:], [[0, JJ], [O, BB], [1, O]]),
                )

                # -- s matmul: accumulate over jj --
                s_ps = ps_pool.tile([BB * O, OD], F32, name="s_ps")
                for jj in range(JJ):
                    nc.tensor.matmul(
                        s_ps, lhsT=cbd[:, jj, :, :], rhs=u_hat[:, jj, :],
                        start=(jj == 0), stop=(jj == JJ - 1),
                    )

                # -- diag extract: s80[(b,o), d] = s_ps[(b,o), o*16+d] --
                sdm = small.tile([O * BB, OD], F32, tag="sdm", name="sdm")
                nc.vector.tensor_mul(sdm, s_ps, maskd[: O * BB, :])
                s80 = small.tile([O * BB, D], F32, tag="s80", name="s80")
                nc.vector.reduce_sum(
                    out=s80,
                    in_=sdm[:].rearrange("p (o d) -> p d o", d=D),
                    axis=mybir.AxisListType.X,
                )
                # squash on [(b,o), d] with per-partition scalars
                nsq = small.tile([O * BB, 1], F32, tag="nsq80", name="nsq")
                sq = small.tile([O * BB, D], F32, tag="sq80", name="sq")
                nc.scalar.square(sq, s80)
                nc.vector.reduce_sum(
                    out=nsq, in_=sq, axis=mybir.AxisListType.X
                )
                rt = small.tile([O * BB, 1], F32, tag="rt80", name="rt")
                nc.scalar.activation(
                    rt, nsq, mybir.ActivationFunctionType.Ln
                )
                nc.scalar.activation(
                    rt, rt, mybir.ActivationFunctionType.Exp, scale=0.5
                )
                op1 = small.tile([O * BB, 1], F32, tag="op180", name="op1")
                nc.vector.tensor_scalar_add(op1, nsq, 1.0)
                rec80 = small.tile([O * BB, 1], F32, tag="rec80", name="rec80")
                nc.vector.reciprocal(rec80, op1)
                nc.vector.tensor_mul(rec80, rec80, rt)
                vcur = small.tile([O * BB, D], F32, tag="vcur", name="vcur")
                nc.vector.tensor_scalar_mul(vcur, s80, rec80)
                st["vcur"] = vcur

                if t == N_ITERS - 1:
                    # v_out[blk*8+b, o*16+d] = vcur[b*10+o, d] (same order)
                    nc.sync.dma_start(
                        out=out_d[blk * BB : (blk + 1) * BB, :], in_=vcur
                    )

            # software pipeline: stage s emits build(s), then both routes'
            # agreement phases (DVE scans + Pool tree) before either tail
            # (softmax/cbd/s-matmul/squash), so ready scans are never stuck
            # behind a tail blocked on the other engine's agreement.
            compute_v0()
            for s in range(NBLK + 1):
                if s < NBLK:
                    build(s)
                if s < NBLK:
                    route_agree(s, 1)
                if 1 <= s:
                    route_agree(s - 1, 2)
                if s < NBLK:
                    route_tail(s, 1)
                if 1 <= s:
                    route_tail(s - 1, 2)
    nc.compile()
    return nc


# ---------------- host side ----------------

_NC_CACHE = None


def _get_nc():
    global _NC_CACHE
    if _NC_CACHE is None:
        _NC_CACHE = build_program()
    return _NC_CACHE


def _pack_wr(W):
    # Wr[g*8+k, jj*160 + o*16 + d] = W[jj*16+g, o, d, k]
    return np.ascontiguousarray(
        W.reshape(JJ, G, O, D, K).transpose(1, 4, 0, 2, 3).reshape(128, JJ * OD)
    ).astype(np.float16)


def _pack_ut(u_loc):
    # uT[g*8+k, jj*B + b] = u_loc[b, jj*16+g, k]
    return np.ascontiguousarray(
        u_loc.reshape(B, JJ, G, K).transpose(2, 3, 1, 0).reshape(128, JJ * B)
    ).astype(np.float16)


def _mask_b():
    # maskb[(b,g), (b',o)] = (b' == b), f16
    p = np.arange(128)
    mb = (np.arange(BB)[None, :] == (p // G)[:, None]).astype(np.float16)
    return np.ascontiguousarray(np.repeat(mb, O, axis=1))


def _mask_d():
    # maskd[(b,o) p<80, o'*16+d] = (o' == o); rows >=80 zero
    md = np.zeros((128, OD), dtype=np.float32)
    po = np.arange(O * BB) % O
    for od in range(OD):
        md[: O * BB, od] = (od // D == po).astype(np.float32)
    return md


def _pack_bdu(u_loc):
    # bdu[(blk,ch)*128 + g*8+k, (j, b, g')] = u_loc[blk*8+b, (ch*9+j)*16+g', k]
    #   nonzero only when g' == g; contiguous per (blk, ch) slice.
    u4 = u_loc.reshape(NBLK, BB, JJ // 9, 9, G, K)  # (blk, b, ch, j, g, k)
    out = np.zeros((NBLK, 8, G, K, 9, BB, G), dtype=np.float16)
    for g in range(G):
        # (blk, ch, k, j, b)
        out[:, :, g, :, :, :, g] = u4[:, :, :, :, g, :].transpose(0, 2, 4, 3, 1)
    return np.ascontiguousarray(out.reshape(NBLK * 8 * 128, 9 * BB * G))


LAST_RESULTS = None


def kernel(u, W):
    from concourse.bass_utils import run_bass_kernel_spmd

    global LAST_RESULTS
    u = np.asarray(u, dtype=np.float32)
    W = np.asarray(W, dtype=np.float32)
    nc = _get_nc()
    wr = _pack_wr(W)
    mb = _mask_b()
    md = _mask_d()
    in_maps = []
    for c in range(8):
        u_loc = u[c * B : (c + 1) * B]
        in_maps.append(
            {
                "wr": wr,
                "ut": _pack_ut(u_loc),
                "bdu": _pack_bdu(u_loc),
                "maskb": mb,
                "maskd": md,
            }
        )
    trace = bool(int(os.environ.get("KBENCH_TRACE", "0")))
    try:
        res = run_bass_kernel_spmd(
            nc, in_maps, core_ids=list(range(8)), trace=trace
        )
    except ModuleNotFoundError:
        # axon NTFF hook unavailable in this container; run without trace
        res = run_bass_kernel_spmd(nc, in_maps, core_ids=list(range(8)))
    LAST_RESULTS = res
    outs = [r["v_out"].reshape(B, O, D) for r in res.results]
    return np.concatenate(outs, axis=0).astype(np.float32)


# revision 25
# speedup vs baseline: 2.2685x; 1.1413x over previous
"""CapsuleLayer (dynamic routing) Trainium2 kernel.

Self-contained: shards the full inputs over 8 NeuronCores (data-parallel over
batch), runs a Bass/Tile kernel per core, gathers the full output.

Shapes (full): u [256, 1152, 8] f32, W [1152, 10, 16, 8] f32 -> v [256, 10, 16].
Per core: B=32 batches, W replicated.

Math (per core, ROUTING_ITERS=3):
  u_hat[b,i,od] = sum_k W[i,od,k] * u[b,i,k]          (od = o*16+d)
  b0 = 0; for t in 0..2: c = softmax(b, o); s = sum_i c*u_hat; v = squash(s);
  if t<2: b += sum_d u_hat*v

Device layouts (i = jj*16+g, jj<72, g<16; partitions in [.]):
  Wr  [(g,k)=128, (jj,od)=11520]   f16 (host-pretransposed W)
  uT  [(g,k)=128, (jj,b)=2304]     f16 (host-pretransposed u shard)
  BDu [(g,k)=128, (jj,b8,g')]      f16 block-diag u, host-packed, DMA-streamed
  u_hat [(b8,g16)=128, (jj,od)]    f16, built by PE: BDu.T @ Wr per 8-batch blk
  s matmul: lhsT = block-diag c (f16, built by 4x-mode partition-slice copies
  into a memset-once tile), rhs = u_hat -> psum[(b',o),od] f32
  t=0 needs no per-blk s pass: v0 comes from the dense ut@wr contraction.
  Agreement sum_d u_hat*v: jj 0..49 on DVE (fused scan-MAC + strided diff),
  jj 50..71 on Pool (mul + 4-level tree reduce). Blocks are software-pipelined:
  stage s emits build(s), route(s-2, t2), route(s-1, t1).
"""

import os
import sys

import numpy as np

for _p in ("/opt/trn_rl_repo", "/root/.axon_site/_ro/trn_rl_repo"):
    if os.path.isdir(_p) and _p not in sys.path:
        sys.path.insert(0, _p)

import concourse.bacc as bacc
import concourse.bass as bass
import concourse.mybir as mybir
import concourse.tile as tile

F32 = mybir.dt.float32
F16 = mybir.dt.float16


def _register_scan_mac():
    """Custom DVE op: out[p,k] = cumsum_k(in0*in1) (fp32 state)."""
    import numpy as np

    from concourse import dve_ops as dops
    from concourse.dve_spec import AluOp, Spec, Src0, Src1, lower, scan
    from concourse.dve_uop import DveOpSpec

    name = "SCAN_MAC_ANT"
    for op in dops.OPS:
        if op.name == name:
            return op
    spec = Spec(
        body=scan(AluOp.ADD, Src0 * Src1),
        reference=lambda in0, in1, c0, c1, c2: np.cumsum(
            np.asarray(in0, np.float32).reshape(in0.shape[0], -1)
            * np.asarray(in1, np.float32).reshape(in1.shape[0], -1),
            axis=-1,
        ).reshape(in0.shape),
    )
    shas = {}
    for ver in ("v3", "v4"):
        uops = lower(spec, ver=ver)
        shas[ver] = DveOpSpec(
            name=name, opcode=0, uops=uops, rd1_en=True
        ).sha(ver)
    op = dops.DveOp(name, spec, subdim=False, uops_sha=shas)
    dops.OPS.append(op)
    dops.CUSTOM_DVE_SPECS[name] = spec
    dops._SUB_OPCODE_FOR_NAME[name] = dops._CUSTOM_DVE_ROW_BASE + len(dops.OPS) - 1
    assert dops._SUB_OPCODE_FOR_NAME[name] < 0x20
    return op


_SCAN_MAC = _register_scan_mac()

# Problem constants (per core)
B = 32          # local batch (256 / 8 cores)
I = 1152        # in capsules
O = 10          # out capsules
D = 16          # out dim
K = 8           # in dim
JJ = 72         # i groups of 16
G = 16          # group size
OD = O * D      # 160
BB = 8          # batch block (psum/output partition packing)
NBLK = B // BB  # 4
N_ITERS = 3
NDVE = 2        # scan chunks per (blk,t) on DVE
AC_DEF = 26     # jj per DVE agreement scan chunk
AC_LAST = 29    # the drain route (last blk, t2) is serial: DVE-heavy split
AC_MAX = 29


def _ap(base, free_dims, extra_offset=0):
    """AP with the base's partition dim and explicit free [step, count] dims."""
    return bass.AP(
        tensor=base.tensor,
        offset=base.offset + extra_offset,
        ap=[list(base.ap[0])] + [list(d) for d in free_dims],
    )


def _squash(nc, pool, s_sb, p, v_out):
    """squash over d (16) per o segment. s_sb: [p, 160] f32 sbuf -> v_out."""
    sq = pool.tile([p, OD], F32, tag="sq")
    nc.scalar.square(sq, s_sb)
    nsq = pool.tile([p, O], F32, tag="nsq")
    nc.vector.reduce_sum(
        out=nsq, in_=sq[:].rearrange("p (o d) -> p o d", d=D),
        axis=mybir.AxisListType.X,
    )
    # sqrt(x) = exp(0.5*ln(x)) — keeps ACT on one table set (ln/exp)
    rt = pool.tile([p, O], F32, tag="rt")
    nc.scalar.activation(rt, nsq, mybir.ActivationFunctionType.Ln)
    nc.scalar.activation(rt, rt, mybir.ActivationFunctionType.Exp, scale=0.5)
    nc.vector.tensor_scalar_add(rt, rt, 1e-8)     # + eps
    op1 = pool.tile([p, O], F32, tag="op1")
    nc.vector.tensor_scalar_add(op1, nsq, 1.0)    # 1 + |s|^2
    nc.vector.tensor_mul(op1, op1, rt)            # (1+n)(sqrt+eps)
    rec = pool.tile([p, O], F32, tag="rec")
    nc.vector.reciprocal(rec, op1)
    nc.vector.tensor_mul(rec, rec, nsq)           # n/((1+n)(sqrt+eps))
    nc.vector.tensor_mul(
        v_out[:].rearrange("p (o d) -> p o d", d=D),
        s_sb[:].rearrange("p (o d) -> p o d", d=D),
        _ap(rec[:], [[1, O], [0, D]]),
    )
    return v_out


def _pin_act_table():
    """Make every ACT function we use resolve to the one set containing all
    of them (natural_log_exp_and_others), so bacc hoists a single
    InstLoadActFuncSet instead of thrashing Exp<->Ln sets (~1.3us/load)."""
    from concourse.bacc import get_activation_tables

    tabs = get_activation_tables("gen3")
    keep = "natural_log_exp_and_others"
    if keep not in tabs:
        return
    ours = {
        mybir.ActivationFunctionType.Exp,
        mybir.ActivationFunctionType.Ln,
        mybir.ActivationFunctionType.Square,
        mybir.ActivationFunctionType.Copy,
        mybir.ActivationFunctionType.Identity,
    }
    if not ours <= tabs[keep]:
        return
    for name, s in tabs.items():
        if name != keep:
            s -= ours


def build_program():
    _pin_act_table()
    nc = bacc.Bacc("TRN2")
    wr_d = nc.dram_tensor("wr", [128, JJ * OD], F16, kind="ExternalInput")
    ut_d = nc.dram_tensor("ut", [128, JJ * B], F16, kind="ExternalInput")
    # block-diag u, host-packed contiguous per (blk, ch): [4*8*128, 1152]
    bdu_d = nc.dram_tensor(
        "bdu", [NBLK * 8 * 128, 9 * BB * G], F16, kind="ExternalInput"
    )
    mb_d = nc.dram_tensor("maskb", [128, BB * O], F16, kind="ExternalInput")
    md_d = nc.dram_tensor("maskd", [128, OD], F32, kind="ExternalInput")
    out_d = nc.dram_tensor("v_out", [B, OD], F32, kind="ExternalOutput")

    with tile.TileContext(nc) as tc:
        with (
            tc.tile_pool(name="persist", bufs=1) as persist,
            tc.tile_pool(name="uhat", bufs=4) as uhat_pool,
            tc.tile_pool(name="bdu", bufs=2) as bdu_pool,
            tc.tile_pool(name="pp", bufs=1) as pp_pool,
            tc.tile_pool(name="blog", bufs=2) as blog_pool,
            tc.tile_pool(name="cbuf", bufs=2) as cbuf_pool,
            tc.tile_pool(name="small", bufs=3) as small,
            tc.tile_pool(name="pb", bufs=4, space="PSUM") as pb_pool,
            tc.tile_pool(name="ps", bufs=2, space="PSUM") as ps_pool,
            tc.tile_pool(name="ps0", bufs=1, space="PSUM") as ps0_pool,
        ):
            # ---- resident loads (bulk on the Act HWDGE queue) ----
            ut = persist.tile([128, JJ, B], F16)
            nc.scalar.dma_start(
                out=ut, in_=ut_d[:].rearrange("p (a b) -> p a b", b=B)
            )
            wr = persist.tile([128, JJ, OD], F16)

            def load_wr(ch):
                nc.scalar.dma_start(
                    out=wr[:, ch * 9 : (ch + 1) * 9, :],
                    in_=wr_d[:, ch * 9 * OD : (ch + 1) * 9 * OD].rearrange(
                        "p (a b) -> p a b", b=OD
                    ),
                )

            for _ch in range(8):
                load_wr(_ch)
            maskb = persist.tile([128, BB * O], F16)
            nc.sync.dma_start(out=maskb, in_=mb_d[:])
            maskd = persist.tile([128, OD], F32)
            nc.sync.dma_start(out=maskd, in_=md_d[:])

            # scan scratch: [128, 1+AC*OD] f32, col 0 pinned to zero so the
            # per-segment sums are single strided subs (S[16n+16]-S[16n]).
            scr = persist.tile([128, 1 + AC_MAX * OD], F16)
            nc.vector.memset(scr[:, 0:1], 0.0)
            # block-diag c lhsT tiles, rebuilt by masked multiply per route
            cbd_t = [persist.tile([128, JJ, BB, O], F16, tag=f"cbd{i}",
                                  name=f"cbd{i}")
                     for i in range(2)]

            v0 = persist.tile([B, OD], F32, tag="v0")

            def compute_v0():
                # s0 = 0.1 * sum_i u_hat  (dense (i,k) contraction)
                s0_ps = ps0_pool.tile([B, OD], F32, name="s0_ps")
                for jj in range(JJ):
                    nc.tensor.matmul(
                        s0_ps, lhsT=ut[:, jj, :], rhs=wr[:, jj, :],
                        start=(jj == 0), stop=(jj == JJ - 1),
                    )
                s0_sb = small.tile([B, OD], F32, tag="s0", name="s0_sb")
                nc.scalar.activation(
                    s0_sb, s0_ps, mybir.ActivationFunctionType.Copy, scale=0.1
                )
                _squash(nc, small, s0_sb, B, v0)  # [32, 160]

            state = {}  # blk -> dict(u_hat, blog, blog2, vcur)
            route_n = [0]  # tail emission counter (cbd buffer parity)

            def build(blk):
                u_hat = uhat_pool.tile([128, JJ, OD], F16, name="u_hat")
                state[blk] = {"u_hat": u_hat}
                for ch in range(8):  # 9 jj per chunk
                    bdu = bdu_pool.tile([128, 9, BB, G], F16, name="bdu")
                    nc.sync.dma_start(
                        out=bdu,
                        in_=bdu_d[
                            (blk * 8 + ch) * 128 : (blk * 8 + ch + 1) * 128, :
                        ].rearrange("p (a b g) -> p a b g", b=BB, g=G),
                    )

                    for j3 in range(3):  # 3-jj groups share one psum bank
                        ps = pb_pool.tile([128, 3, OD], F32, name="ps")
                        for j in range(3):
                            jj = ch * 9 + j3 * 3 + j
                            nc.tensor.matmul(
                                ps[:, j, :], lhsT=bdu[:, j3 * 3 + j, :, :],
                                rhs=wr[:, jj, :], start=True, stop=True,
                            )
                        jj0 = ch * 9 + j3 * 3
                        nc.scalar.copy(u_hat[:, jj0 : jj0 + 3, :], ps)

            def route_agree(blk, t):
                st = state[blk]
                u_hat = st["u_hat"]
                # -- vrep[(b,g), od] = v_{t-1}[b, od] --
                vrep = small.tile([128, OD], F32, tag="vrep", name="vrep")
                if t == 1:
                    src = _ap(v0[:], [[0, G], [1, OD]])
                    src = bass.AP(
                        tensor=src.tensor,
                        offset=src.offset + blk * BB * v0[:].ap[0][0],
                        ap=[[v0[:].ap[0][0], BB]] + list(src.ap)[1:],
                    )
                else:
                    vtmp = small.tile([BB, OD], F32, tag="vtmp", name="vtmp",
                                      bufs=4)
                    nc.scalar.dma_start(out=vtmp, in_=st["vcur"])
                    vt = vtmp[:]
                    src = bass.AP(
                        tensor=vt.tensor, offset=vt.offset,
                        ap=[[vt.ap[0][0], BB], [0, G], [1, OD]],
                    )
                nc.scalar.dma_start(out=vrep, in_=src)

                # -- agreement a[b,i,o] = sum_d u_hat*v into bl --
                if t == 1:
                    bl = blog_pool.tile([128, JJ, O], F32, tag="blog",
                                        name="blog", bufs=3)
                    st["blog"] = bl
                else:
                    bl = blog_pool.tile([128, JJ, O], F32, tag="blog2",
                                        name="blog2", bufs=1)
                AC = AC_LAST if (blk == NBLK - 1 and t == 2) else AC_DEF
                PJJ = JJ - NDVE * AC
                # DVE chunks: fused scan-MAC cumsum, segment sums by
                # strided subtraction against the zero-led scratch.
                for h in range(NDVE):
                    nc.vector._custom_dve(
                        _SCAN_MAC,
                        out=scr[:, 1 : 1 + AC * OD],
                        in0=u_hat[:, h * AC : (h + 1) * AC, :],
                        in1=_ap(vrep[:], [[0, AC], [1, OD]]),
                    )
                    sv = scr[:]
                    nseg = AC * O
                    s_hi = _ap(sv, [[D, nseg]], extra_offset=D)
                    s_lo = _ap(sv, [[D, nseg]], extra_offset=0)
                    blf = bl[:, h * AC : (h + 1) * AC, :].rearrange(
                        "p a o -> p (a o)"
                    )
                    nc.vector.tensor_sub(blf, s_hi, s_lo)
                # Pool chunk: products then 4-level tree reduce over d.
                jp = NDVE * AC
                nsegp = PJJ * O
                pp = pp_pool.tile([128, PJJ * OD], F16, name="pp", bufs=2)
                nc.gpsimd.tensor_mul(
                    pp,
                    u_hat[:, jp:, :].rearrange("p a od -> p (a od)"),
                    _ap(vrep[:], [[0, PJJ], [1, OD]]),
                )
                tr1 = pp_pool.tile([128, nsegp * 8], F16, tag="tr1",
                                   name="tr1", bufs=1)
                nc.gpsimd.tensor_add(
                    tr1[:].rearrange("p (s d) -> p s d", d=8),
                    _ap(pp[:], [[D, nsegp], [1, 8]]),
                    _ap(pp[:], [[D, nsegp], [1, 8]], extra_offset=8),
                )
                tr2 = pp_pool.tile([128, nsegp * 4], F16, tag="tr2",
                                   name="tr2", bufs=1)
                nc.gpsimd.tensor_add(
                    tr2[:].rearrange("p (s d) -> p s d", d=4),
                    _ap(tr1[:], [[8, nsegp], [1, 4]]),
                    _ap(tr1[:], [[8, nsegp], [1, 4]], extra_offset=4),
                )
                tr3 = pp_pool.tile([128, nsegp * 2], F16, tag="tr3",
                                   name="tr3", bufs=1)
                nc.gpsimd.tensor_add(
                    tr3[:].rearrange("p (s d) -> p s d", d=2),
                    _ap(tr2[:], [[4, nsegp], [1, 2]]),
                    _ap(tr2[:], [[4, nsegp], [1, 2]], extra_offset=2),
                )
                blfp = bl[:, jp:, :].rearrange("p a o -> p (a o)")
                nc.gpsimd.tensor_add(
                    blfp,
                    _ap(tr3[:], [[2, nsegp]]),
                    _ap(tr3[:], [[2, nsegp]], extra_offset=1),
                )

                st["bl"] = bl

            def route_tail(blk, t):
                st = state[blk]
                u_hat = st["u_hat"]
                bl = st["bl"]
                # -- c = softmax(blog[+blog2]) over o --
                if t == 1:
                    blogs = bl
                else:
                    blogs = st["blog"]
                    nc.vector.tensor_add(blogs, blogs, bl)
                cb32 = cbuf_pool.tile([128, JJ, O], F32, tag="cb32",
                                      name="cb32")
                nc.scalar.activation(
                    cb32, blogs, mybir.ActivationFunctionType.Exp
                )
                ssum = small.tile([128, JJ], F32, tag="ssum", name="ssum")
                nc.vector.reduce_sum(
                    out=ssum, in_=cb32, axis=mybir.AxisListType.X
                )
                rec = small.tile([128, JJ], F32, tag="srec", name="srec")
                nc.vector.reciprocal(rec, ssum)
                cb2 = cbuf_pool.tile([128, JJ, O], F16, tag="cb2", name="cb2")
                nc.vector.tensor_mul(
                    cb2, cb32, _ap(rec[:], [[1, JJ], [0, O]])
                )

                # -- cbd = cb2 (bcast over b') * maskb (bcast over jj) --
                # all-f16 packed SBUF operands -> DVE 2x mode; two jj-halves
                # so the s-matmul can start on the first half earlier
                cbd = cbd_t[route_n[0] % 2]
                route_n[0] += 1
                JH = JJ // 2
                for half in range(2):
                    nc.vector.tensor_mul(
                        cbd[:, half * JH : (half + 1) * JH, :, :],
                        _ap(cb2[:], [[O, JH], [0, BB], [1, O]],
                            extra_offset=half * JH * O),
                        _ap(maskb[:], [[0, JH], [O, BB], [1, O]]),
                    )

                # -- s matmul: accumulate over jj --
                s_ps = ps_pool.tile([BB * O, OD], F32, name="s_ps")
                for jj in range(JJ):
                    nc.tensor.matmul(
                        s_ps, lhsT=cbd[:, jj, :, :], rhs=u_hat[:, jj, :],
                        start=(jj == 0), stop=(jj == JJ - 1),
                    )

                # -- diag extract: s80[(b,o), d] = s_ps[(b,o), o*16+d] --
                sdm = small.tile([O * BB, OD], F32, tag="sdm", name="sdm")
                nc.vector.tensor_mul(sdm, s_ps, maskd[: O * BB, :])
                s80 = small.tile([O * BB, D], F32, tag="s80", name="s80")
                nc.vector.reduce_sum(
                    out=s80,
                    in_=sdm[:].rearrange("p (o d) -> p d o", d=D),
                    axis=mybir.AxisListType.X,
                )
                # squash on [(b,o), d] with per-partition scalars
                nsq = small.tile([O * BB, 1], F32, tag="nsq80", name="nsq")
                sq = small.tile([O * BB, D], F32, tag="sq80", name="sq")
                nc.scalar.square(sq, s80)
                nc.vector.reduce_sum(
                    out=nsq, in_=sq, axis=mybir.AxisListType.X
                )
                rt = small.tile([O * BB, 1], F32, tag="rt80", name="rt")
                nc.scalar.activation(
                    rt, nsq, mybir.ActivationFunctionType.Ln
                )
                nc.scalar.activation(
                    rt, rt, mybir.ActivationFunctionType.Exp, scale=0.5
                )
                op1 = small.tile([O * BB, 1], F32, tag="op180", name="op1")
                nc.vector.tensor_scalar_add(op1, nsq, 1.0)
                rec80 = small.tile([O * BB, 1], F32, tag="rec80", name="rec80")
                nc.vector.reciprocal(rec80, op1)
                nc.vector.tensor_mul(rec80, rec80, rt)
                vcur = small.tile([O * BB, D], F32, tag="vcur", name="vcur",
                                  bufs=4)
                nc.vector.tensor_scalar_mul(vcur, s80, rec80)
                st["vcur"] = vcur

                if t == N_ITERS - 1:
                    # v_out[blk*8+b, o*16+d] = vcur[b*10+o, d] (same order)
                    nc.scalar.dma_start(
                        out=out_d[blk * BB : (blk + 1) * BB, :], in_=vcur
                    )

            # software pipeline: stage s emits build(s), then both routes'
            # agreement phases (DVE scans + Pool tree) before either tail
            # (softmax/cbd/s-matmul/squash), so ready scans are never stuck
            # behind a tail blocked on the other engine's agreement.
            compute_v0()
            for s in range(NBLK + 1):
                if s < NBLK:
                    build(s)
                if s < NBLK:
                    route_agree(s, 1)
                if s == NBLK:
                    t2s = [NBLK - 2, NBLK - 1]  # drain: both remaining t2s
                elif 2 <= s:
                    t2s = [s - 2]
                else:
                    t2s = []
                for n in t2s:
                    route_agree(n, 2)
                if s < NBLK:
                    route_tail(s, 1)
                for n in t2s:
                    route_tail(n, 2)
    nc.compile()
    return nc


# ---------------- host side ----------------

_NC_CACHE = None


def _get_nc():
    global _NC_CACHE
    if _NC_CACHE is None:
        _NC_CACHE = build_program()
    return _NC_CACHE


def _pack_wr(W):
    # Wr[g*8+k, jj*160 + o*16 + d] = W[jj*16+g, o, d, k]
    return np.ascontiguousarray(
        W.reshape(JJ, G, O, D, K).transpose(1, 4, 0, 2, 3).reshape(128, JJ * OD)
    ).astype(np.float16)


def _pack_ut(u_loc):
    # uT[g*8+k, jj*B + b] = u_loc[b, jj*16+g, k]
    return np.ascontiguousarray(
        u_loc.reshape(B, JJ, G, K).transpose(2, 3, 1, 0).reshape(128, JJ * B)
    ).astype(np.float16)


def _mask_b():
    # maskb[(b,g), (b',o)] = (b' == b), f16
    p = np.arange(128)
    mb = (np.arange(BB)[None, :] == (p // G)[:, None]).astype(np.float16)
    return np.ascontiguousarray(np.repeat(mb, O, axis=1))


def _mask_d():
    # maskd[(b,o) p<80, o'*16+d] = (o' == o); rows >=80 zero
    md = np.zeros((128, OD), dtype=np.float32)
    po = np.arange(O * BB) % O
    for od in range(OD):
        md[: O * BB, od] = (od // D == po).astype(np.float32)
    return md


def _pack_bdu(u_loc):
    # bdu[(blk,ch)*128 + g*8+k, (j, b, g')] = u_loc[blk*8+b, (ch*9+j)*16+g', k]
    #   nonzero only when g' == g; contiguous per (blk, ch) slice.
    u4 = u_loc.reshape(NBLK, BB, JJ // 9, 9, G, K)  # (blk, b, ch, j, g, k)
    out = np.zeros((NBLK, 8, G, K, 9, BB, G), dtype=np.float16)
    for g in range(G):
        # (blk, ch, k, j, b)
        out[:, :, g, :, :, :, g] = u4[:, :, :, :, g, :].transpose(0, 2, 4, 3, 1)
    return np.ascontiguousarray(out.reshape(NBLK * 8 * 128, 9 * BB * G))


LAST_RESULTS = None


def kernel(u, W):
    from concourse.bass_utils import run_bass_kernel_spmd

    global LAST_RESULTS
    u = np.asarray(u, dtype=np.float32)
    W = np.asarray(W, dtype=np.float32)
    nc = _get_nc()
    wr = _pack_wr(W)
    mb = _mask_b()
    md = _mask_d()
    in_maps = []
    for c in range(8):
        u_loc = u[c * B : (c + 1) * B]
        in_maps.append(
            {
                "wr": wr,
                "ut": _pack_ut(u_loc),
                "bdu": _pack_bdu(u_loc),
                "maskb": mb,
                "maskd": md,
            }
        )
    trace = bool(int(os.environ.get("KBENCH_TRACE", "0")))
    try:
        res = run_bass_kernel_spmd(
            nc, in_maps, core_ids=list(range(8)), trace=trace
        )
    except ModuleNotFoundError:
        # axon NTFF hook unavailable in this container; run without trace
        res = run_bass_kernel_spmd(nc, in_maps, core_ids=list(range(8)))
    LAST_RESULTS = res
    outs = [r["v_out"].reshape(B, O, D) for r in res.results]
    return np.concatenate(outs, axis=0).astype(np.float32)


# revision 46
# speedup vs baseline: 2.3990x; 1.0575x over previous
"""CapsuleLayer (dynamic routing) Trainium2 kernel.

Self-contained: shards the full inputs over 8 NeuronCores (data-parallel over
batch), runs a Bass/Tile kernel per core, gathers the full output.

Shapes (full): u [256, 1152, 8] f32, W [1152, 10, 16, 8] f32 -> v [256, 10, 16].
Per core: B=32 batches, W replicated.

Math (per core, ROUTING_ITERS=3):
  u_hat[b,i,od] = sum_k W[i,od,k] * u[b,i,k]          (od = o*16+d)
  b0 = 0; for t in 0..2: c = softmax(b, o); s = sum_i c*u_hat; v = squash(s);
  if t<2: b += sum_d u_hat*v

Device layouts (i = jj*16+g, jj<72, g<16; partitions in [.]):
  Wr  [(g,k)=128, (jj,od)=11520]   f16 (host-pretransposed W)
  uT  [(g,k)=128, (jj,b)=2304]     f16 (host-pretransposed u shard)
  BDu [(g,k)=128, (jj,b8,g')]      f16 block-diag u, host-packed, DMA-streamed
  u_hat [(b8,g16)=128, (jj,od)]    f16, built by PE: BDu.T @ Wr per 8-batch blk
  s matmul: lhsT = block-diag c (f16, built by 4x-mode partition-slice copies
  into a memset-once tile), rhs = u_hat -> psum[(b',o),od] f32
  t=0 needs no per-blk s pass: v0 comes from the dense ut@wr contraction.
  Agreement sum_d u_hat*v: jj 0..49 on DVE (fused scan-MAC + strided diff),
  jj 50..71 on Pool (mul + 4-level tree reduce). Blocks are software-pipelined:
  stage s emits build(s), route(s-2, t2), route(s-1, t1).
"""

import os
import sys

import numpy as np

for _p in ("/opt/trn_rl_repo", "/root/.axon_site/_ro/trn_rl_repo"):
    if os.path.isdir(_p) and _p not in sys.path:
        sys.path.insert(0, _p)

import concourse.bacc as bacc
import concourse.bass as bass
import concourse.mybir as mybir
import concourse.tile as tile

F32 = mybir.dt.float32
F16 = mybir.dt.float16


def _register_scan_mac():
    """Custom DVE op: out[p,k] = cumsum_k(in0*in1) (fp32 state)."""
    import numpy as np

    from concourse import dve_ops as dops
    from concourse.dve_spec import AluOp, Spec, Src0, Src1, lower, scan
    from concourse.dve_uop import DveOpSpec

    name = "SCAN_MAC_ANT"
    for op in dops.OPS:
        if op.name == name:
            return op
    spec = Spec(
        body=scan(AluOp.ADD, Src0 * Src1),
        reference=lambda in0, in1, c0, c1, c2: np.cumsum(
            np.asarray(in0, np.float32).reshape(in0.shape[0], -1)
            * np.asarray(in1, np.float32).reshape(in1.shape[0], -1),
            axis=-1,
        ).reshape(in0.shape),
    )
    shas = {}
    for ver in ("v3", "v4"):
        uops = lower(spec, ver=ver)
        shas[ver] = DveOpSpec(
            name=name, opcode=0, uops=uops, rd1_en=True
        ).sha(ver)
    op = dops.DveOp(name, spec, subdim=False, uops_sha=shas)
    dops.OPS.append(op)
    dops.CUSTOM_DVE_SPECS[name] = spec
    dops._SUB_OPCODE_FOR_NAME[name] = dops._CUSTOM_DVE_ROW_BASE + len(dops.OPS) - 1
    assert dops._SUB_OPCODE_FOR_NAME[name] < 0x20
    return op


_SCAN_MAC = _register_scan_mac()

# Problem constants (per core)
B = 32          # local batch (256 / 8 cores)
I = 1152        # in capsules
O = 10          # out capsules
D = 16          # out dim
K = 8           # in dim
JJ = 72         # i groups of 16
G = 16          # group size
OD = O * D      # 160
BB = 8          # batch block (psum/output partition packing)
NBLK = B // BB  # 4
N_ITERS = 3
NDVE = 2        # scan chunks per (blk,t) on DVE
AC_DEF = 28     # jj per DVE agreement scan chunk
AC_LAST = 30    # the drain route (last blk, t2) is serial: DVE-heavy split
AC_MAX = 30


def _ap(base, free_dims, extra_offset=0):
    """AP with the base's partition dim and explicit free [step, count] dims."""
    return bass.AP(
        tensor=base.tensor,
        offset=base.offset + extra_offset,
        ap=[list(base.ap[0])] + [list(d) for d in free_dims],
    )


def _squash(nc, pool, s_sb, p, v_out):
    """squash over d (16) per o segment. s_sb: [p, 160] f32 sbuf -> v_out."""
    sq = pool.tile([p, OD], F32, tag="sq", bufs=1)
    nc.scalar.square(sq, s_sb)
    nsq = pool.tile([p, O], F32, tag="nsq", bufs=1)
    nc.vector.reduce_sum(
        out=nsq, in_=sq[:].rearrange("p (o d) -> p o d", d=D),
        axis=mybir.AxisListType.X,
    )
    # sqrt(x) = exp(0.5*ln(x)) — keeps ACT on one table set (ln/exp)
    rt = pool.tile([p, O], F32, tag="rt", bufs=1)
    nc.scalar.activation(rt, nsq, mybir.ActivationFunctionType.Ln)
    nc.scalar.activation(rt, rt, mybir.ActivationFunctionType.Exp, scale=0.5)
    nc.vector.tensor_scalar_add(rt, rt, 1e-8)     # + eps
    op1 = pool.tile([p, O], F32, tag="op1", bufs=1)
    nc.vector.tensor_scalar_add(op1, nsq, 1.0)    # 1 + |s|^2
    nc.vector.tensor_mul(op1, op1, rt)            # (1+n)(sqrt+eps)
    rec = pool.tile([p, O], F32, tag="rec", bufs=1)
    nc.vector.reciprocal(rec, op1)
    nc.vector.tensor_mul(rec, rec, nsq)           # n/((1+n)(sqrt+eps))
    nc.vector.tensor_mul(
        v_out[:].rearrange("p (o d) -> p o d", d=D),
        s_sb[:].rearrange("p (o d) -> p o d", d=D),
        _ap(rec[:], [[1, O], [0, D]]),
    )
    return v_out


def _pin_act_table():
    """Make every ACT function we use resolve to the one set containing all
    of them (natural_log_exp_and_others), so bacc hoists a single
    InstLoadActFuncSet instead of thrashing Exp<->Ln sets (~1.3us/load)."""
    from concourse.bacc import get_activation_tables

    tabs = get_activation_tables("gen3")
    keep = "natural_log_exp_and_others"
    if keep not in tabs:
        return
    ours = {
        mybir.ActivationFunctionType.Exp,
        mybir.ActivationFunctionType.Ln,
        mybir.ActivationFunctionType.Square,
        mybir.ActivationFunctionType.Copy,
        mybir.ActivationFunctionType.Identity,
    }
    if not ours <= tabs[keep]:
        return
    for name, s in tabs.items():
        if name != keep:
            s -= ours


def build_program():
    _pin_act_table()
    nc = bacc.Bacc("TRN2")
    wr_d = nc.dram_tensor("wr", [128, JJ * OD], F16, kind="ExternalInput")
    ut_d = nc.dram_tensor("ut", [128, JJ * B], F16, kind="ExternalInput")
    # block-diag u, host-packed contiguous per (blk, ch): [4*8*128, 1152]
    bdu_d = nc.dram_tensor(
        "bdu", [NBLK * 8 * 128, 9 * BB * G], F16, kind="ExternalInput"
    )
    mb_d = nc.dram_tensor("maskb", [128, BB * O], F16, kind="ExternalInput")
    md_d = nc.dram_tensor("maskd", [128, OD], F32, kind="ExternalInput")
    out_d = nc.dram_tensor("v_out", [B, OD], F32, kind="ExternalOutput")

    with tile.TileContext(nc) as tc:
        with (
            tc.tile_pool(name="persist", bufs=1) as persist,
            tc.tile_pool(name="uhat", bufs=4) as uhat_pool,
            tc.tile_pool(name="bdu", bufs=2) as bdu_pool,
            tc.tile_pool(name="pp", bufs=1) as pp_pool,
            tc.tile_pool(name="blog", bufs=2) as blog_pool,
            tc.tile_pool(name="cbuf", bufs=2) as cbuf_pool,
            tc.tile_pool(name="small", bufs=3) as small,
            tc.tile_pool(name="pb", bufs=5, space="PSUM") as pb_pool,
            tc.tile_pool(name="ps", bufs=2, space="PSUM") as ps_pool,
            tc.tile_pool(name="ps0", bufs=1, space="PSUM") as ps0_pool,
        ):
            # ---- resident loads (bulk on the Act HWDGE queue) ----
            ut = persist.tile([128, JJ, B], F16)
            nc.scalar.dma_start(
                out=ut, in_=ut_d[:].rearrange("p (a b) -> p a b", b=B)
            )
            wr = persist.tile([128, JJ, OD], F16)

            def load_wr(ch):
                nc.scalar.dma_start(
                    out=wr[:, ch * 9 : (ch + 1) * 9, :],
                    in_=wr_d[:, ch * 9 * OD : (ch + 1) * 9 * OD].rearrange(
                        "p (a b) -> p a b", b=OD
                    ),
                )

            for _ch in range(8):
                load_wr(_ch)
            maskb = persist.tile([128, BB * O], F16)
            nc.sync.dma_start(out=maskb, in_=mb_d[:])
            maskd = persist.tile([128, OD], F32)
            nc.sync.dma_start(out=maskd, in_=md_d[:])

            # scan scratch: [128, 1+AC*OD] f32, col 0 pinned to zero so the
            # per-segment sums are single strided subs (S[16n+16]-S[16n]).
            scr = persist.tile([128, 1 + AC_MAX * OD], F16)
            nc.vector.memset(scr[:, 0:1], 0.0)
            # block-diag c lhsT tiles, rebuilt by masked multiply per route
            cbd_t = [persist.tile([128, JJ, BB, O], F16, tag=f"cbd{i}",
                                  name=f"cbd{i}")
                     for i in range(2)]

            v0 = persist.tile([B, OD], F32, tag="v0")

            def compute_v0():
                # s0 = 0.1 * sum_i u_hat  (dense (i,k) contraction)
                s0_ps = ps0_pool.tile([B, OD], F32, name="s0_ps")
                for jj in range(JJ):
                    nc.tensor.matmul(
                        s0_ps, lhsT=ut[:, jj, :], rhs=wr[:, jj, :],
                        start=(jj == 0), stop=(jj == JJ - 1),
                    )
                s0_sb = small.tile([B, OD], F32, tag="s0", name="s0_sb")
                nc.scalar.activation(
                    s0_sb, s0_ps, mybir.ActivationFunctionType.Copy, scale=0.1
                )
                _squash(nc, small, s0_sb, B, v0)  # [32, 160]

            state = {}  # blk -> dict(u_hat, blog, blog2, vcur)
            route_n = [0]  # tail emission counter (cbd buffer parity)

            def build(blk):
                u_hat = uhat_pool.tile([128, JJ, OD], F16, name="u_hat")
                state[blk] = {"u_hat": u_hat}
                for ch in range(8):  # 9 jj per chunk
                    bdu = bdu_pool.tile([128, 9, BB, G], F16, name="bdu")
                    nc.sync.dma_start(
                        out=bdu,
                        in_=bdu_d[
                            (blk * 8 + ch) * 128 : (blk * 8 + ch + 1) * 128, :
                        ].rearrange("p (a b g) -> p a b g", b=BB, g=G),
                    )

                    for j3 in range(3):  # 3-jj groups share one psum bank
                        ps = pb_pool.tile([128, 3, OD], F32, name="ps")
                        for j in range(3):
                            jj = ch * 9 + j3 * 3 + j
                            nc.tensor.matmul(
                                ps[:, j, :], lhsT=bdu[:, j3 * 3 + j, :, :],
                                rhs=wr[:, jj, :], start=True, stop=True,
                            )
                        jj0 = ch * 9 + j3 * 3
                        nc.scalar.copy(u_hat[:, jj0 : jj0 + 3, :], ps)

            def route_agree(blk, t):
                st = state[blk]
                u_hat = st["u_hat"]
                # -- vrep[(b,g), od] = v_{t-1}[b, od] --
                vrep = small.tile([128, OD], F32, tag="vrep", name="vrep")
                if t == 1:
                    src = _ap(v0[:], [[0, G], [1, OD]])
                    src = bass.AP(
                        tensor=src.tensor,
                        offset=src.offset + blk * BB * v0[:].ap[0][0],
                        ap=[[v0[:].ap[0][0], BB]] + list(src.ap)[1:],
                    )
                else:
                    vtmp = small.tile([BB, OD], F32, tag="vtmp", name="vtmp",
                                      bufs=4)
                    nc.scalar.dma_start(out=vtmp, in_=st["vcur"])
                    vt = vtmp[:]
                    src = bass.AP(
                        tensor=vt.tensor, offset=vt.offset,
                        ap=[[vt.ap[0][0], BB], [0, G], [1, OD]],
                    )
                nc.scalar.dma_start(out=vrep, in_=src)

                # -- agreement a[b,i,o] = sum_d u_hat*v into bl --
                if t == 1:
                    bl = blog_pool.tile([128, JJ, O], F16, tag="blog",
                                        name="blog", bufs=3)
                    st["blog"] = bl
                else:
                    bl = blog_pool.tile([128, JJ, O], F16, tag="blog2",
                                        name="blog2", bufs=1)
                AC = AC_LAST if (blk >= NBLK - 2 and t == 2) else AC_DEF
                PJJ = JJ - NDVE * AC
                # DVE chunks: fused scan-MAC cumsum, segment sums by
                # strided subtraction against the zero-led scratch.
                for h in range(NDVE):
                    nc.vector._custom_dve(
                        _SCAN_MAC,
                        out=scr[:, 1 : 1 + AC * OD],
                        in0=u_hat[:, h * AC : (h + 1) * AC, :],
                        in1=_ap(vrep[:], [[0, AC], [1, OD]]),
                    )
                    sv = scr[:]
                    nseg = AC * O
                    s_hi = _ap(sv, [[D, nseg]], extra_offset=D)
                    s_lo = _ap(sv, [[D, nseg]], extra_offset=0)
                    blf = bl[:, h * AC : (h + 1) * AC, :].rearrange(
                        "p a o -> p (a o)"
                    )
                    nc.vector.tensor_sub(blf, s_hi, s_lo)
                # Pool chunk: products then 4-level tree reduce over d.
                jp = NDVE * AC
                nsegp = PJJ * O
                pp = pp_pool.tile([128, PJJ * OD], F16, name="pp", bufs=2)
                nc.gpsimd.tensor_mul(
                    pp,
                    u_hat[:, jp:, :].rearrange("p a od -> p (a od)"),
                    _ap(vrep[:], [[0, PJJ], [1, OD]]),
                )
                tr1 = pp_pool.tile([128, nsegp * 8], F16, tag="tr1",
                                   name="tr1", bufs=1)
                nc.gpsimd.tensor_add(
                    tr1[:].rearrange("p (s d) -> p s d", d=8),
                    _ap(pp[:], [[D, nsegp], [1, 8]]),
                    _ap(pp[:], [[D, nsegp], [1, 8]], extra_offset=8),
                )
                tr2 = pp_pool.tile([128, nsegp * 4], F16, tag="tr2",
                                   name="tr2", bufs=1)
                nc.gpsimd.tensor_add(
                    tr2[:].rearrange("p (s d) -> p s d", d=4),
                    _ap(tr1[:], [[8, nsegp], [1, 4]]),
                    _ap(tr1[:], [[8, nsegp], [1, 4]], extra_offset=4),
                )
                tr3 = pp_pool.tile([128, nsegp * 2], F16, tag="tr3",
                                   name="tr3", bufs=1)
                nc.gpsimd.tensor_add(
                    tr3[:].rearrange("p (s d) -> p s d", d=2),
                    _ap(tr2[:], [[4, nsegp], [1, 2]]),
                    _ap(tr2[:], [[4, nsegp], [1, 2]], extra_offset=2),
                )
                blfp = bl[:, jp:, :].rearrange("p a o -> p (a o)")
                nc.gpsimd.tensor_add(
                    blfp,
                    _ap(tr3[:], [[2, nsegp]]),
                    _ap(tr3[:], [[2, nsegp]], extra_offset=1),
                )

                st["bl"] = bl

            def route_tail(blk, t):
                st = state[blk]
                u_hat = st["u_hat"]
                bl = st["bl"]
                # -- c = softmax(blog[+blog2]) over o --
                if t == 1:
                    blogs = bl
                else:
                    blogs = st["blog"]
                    nc.vector.tensor_add(blogs, blogs, bl)
                cb32 = cbuf_pool.tile([128, JJ, O], F32, tag="cb32",
                                      name="cb32")
                nc.scalar.activation(
                    cb32, blogs, mybir.ActivationFunctionType.Exp
                )
                ssum = small.tile([128, JJ], F32, tag="ssum", name="ssum")
                nc.vector.reduce_sum(
                    out=ssum, in_=cb32, axis=mybir.AxisListType.X
                )
                rec = small.tile([128, JJ], F32, tag="srec", name="srec")
                nc.vector.reciprocal(rec, ssum)
                cb2 = cbuf_pool.tile([128, JJ, O], F16, tag="cb2", name="cb2")
                nc.vector.tensor_mul(
                    cb2, cb32, _ap(rec[:], [[1, JJ], [0, O]])
                )

                # -- cbd = cb2 (bcast over b') * maskb (bcast over jj) --
                # all-f16 packed SBUF operands -> DVE 2x mode; two jj-halves
                # so the s-matmul can start on the first half earlier
                cbd = cbd_t[route_n[0] % 2]
                route_n[0] += 1
                JH = JJ // 2
                for half in range(2):
                    nc.vector.tensor_mul(
                        cbd[:, half * JH : (half + 1) * JH, :, :],
                        _ap(cb2[:], [[O, JH], [0, BB], [1, O]],
                            extra_offset=half * JH * O),
                        _ap(maskb[:], [[0, JH], [O, BB], [1, O]]),
                    )

                # -- s matmul: accumulate over jj --
                s_ps = ps_pool.tile([BB * O, OD], F32, name="s_ps")
                for jj in range(JJ):
                    nc.tensor.matmul(
                        s_ps, lhsT=cbd[:, jj, :, :], rhs=u_hat[:, jj, :],
                        start=(jj == 0), stop=(jj == JJ - 1),
                    )

                # -- diag extract: s80[(b,o), d] = s_ps[(b,o), o*16+d] --
                sdm = small.tile([O * BB, OD], F32, tag="sdm", name="sdm")
                nc.vector.tensor_mul(sdm, s_ps, maskd[: O * BB, :])
                s80 = small.tile([O * BB, D], F32, tag="s80", name="s80")
                nc.vector.reduce_sum(
                    out=s80,
                    in_=sdm[:].rearrange("p (o d) -> p d o", d=D),
                    axis=mybir.AxisListType.X,
                )
                # squash on [(b,o), d] with per-partition scalars
                nsq = small.tile([O * BB, 1], F32, tag="nsq80", name="nsq")
                sq = small.tile([O * BB, D], F32, tag="sq80", name="sq")
                nc.scalar.square(sq, s80)
                nc.vector.reduce_sum(
                    out=nsq, in_=sq, axis=mybir.AxisListType.X
                )
                rt = small.tile([O * BB, 1], F32, tag="rt80", name="rt")
                nc.scalar.activation(
                    rt, nsq, mybir.ActivationFunctionType.Ln
                )
                nc.scalar.activation(
                    rt, rt, mybir.ActivationFunctionType.Exp, scale=0.5
                )
                op1 = small.tile([O * BB, 1], F32, tag="op180", name="op1")
                nc.vector.tensor_scalar_add(op1, nsq, 1.0)
                rec80 = small.tile([O * BB, 1], F32, tag="rec80", name="rec80")
                nc.vector.reciprocal(rec80, op1)
                nc.vector.tensor_mul(rec80, rec80, rt)
                vcur = small.tile([O * BB, D], F32, tag="vcur", name="vcur",
                                  bufs=4)
                nc.vector.tensor_scalar_mul(vcur, s80, rec80)
                st["vcur"] = vcur

                if t == N_ITERS - 1:
                    # v_out[blk*8+b, o*16+d] = vcur[b*10+o, d] (same order)
                    nc.scalar.dma_start(
                        out=out_d[blk * BB : (blk + 1) * BB, :], in_=vcur
                    )

            # software pipeline: stage s emits build(s), then both routes'
            # agreement phases (DVE scans + Pool tree) before either tail
            # (softmax/cbd/s-matmul/squash), so ready scans are never stuck
            # behind a tail blocked on the other engine's agreement.
            compute_v0()
            for s in range(NBLK + 1):
                if s < NBLK:
                    build(s)
                if s < NBLK:
                    route_agree(s, 1)
                if s == NBLK:
                    t2s = [NBLK - 2, NBLK - 1]  # drain: both remaining t2s
                elif 2 <= s:
                    t2s = [s - 2]
                else:
                    t2s = []
                for n in t2s:
                    route_agree(n, 2)
                for n in t2s:
                    route_tail(n, 2)
                if s < NBLK:
                    route_tail(s, 1)
    nc.compile()
    return nc


# ---------------- host side ----------------

_NC_CACHE = None


def _get_nc():
    global _NC_CACHE
    if _NC_CACHE is None:
        _NC_CACHE = build_program()
    return _NC_CACHE


def _pack_wr(W):
    # Wr[g*8+k, jj*160 + o*16 + d] = W[jj*16+g, o, d, k]
    return np.ascontiguousarray(
        W.reshape(JJ, G, O, D, K).transpose(1, 4, 0, 2, 3).reshape(128, JJ * OD)
    ).astype(np.float16)


def _pack_ut(u_loc):
    # uT[g*8+k, jj*B + b] = u_loc[b, jj*16+g, k]
    return np.ascontiguousarray(
        u_loc.reshape(B, JJ, G, K).transpose(2, 3, 1, 0).reshape(128, JJ * B)
    ).astype(np.float16)


def _mask_b():
    # maskb[(b,g), (b',o)] = (b' == b), f16
    p = np.arange(128)
    mb = (np.arange(BB)[None, :] == (p // G)[:, None]).astype(np.float16)
    return np.ascontiguousarray(np.repeat(mb, O, axis=1))


def _mask_d():
    # maskd[(b,o) p<80, o'*16+d] = (o' == o); rows >=80 zero
    md = np.zeros((128, OD), dtype=np.float32)
    po = np.arange(O * BB) % O
    for od in range(OD):
        md[: O * BB, od] = (od // D == po).astype(np.float32)
    return md


def _pack_bdu(u_loc):
    # bdu[(blk,ch)*128 + g*8+k, (j, b, g')] = u_loc[blk*8+b, (ch*9+j)*16+g', k]
    #   nonzero only when g' == g; contiguous per (blk, ch) slice.
    u4 = u_loc.reshape(NBLK, BB, JJ // 9, 9, G, K)  # (blk, b, ch, j, g, k)
    out = np.zeros((NBLK, 8, G, K, 9, BB, G), dtype=np.float16)
    for g in range(G):
        # (blk, ch, k, j, b)
        out[:, :, g, :, :, :, g] = u4[:, :, :, :, g, :].transpose(0, 2, 4, 3, 1)
    return np.ascontiguousarray(out.reshape(NBLK * 8 * 128, 9 * BB * G))


LAST_RESULTS = None


def kernel(u, W):
    from concourse.bass_utils import run_bass_kernel_spmd

    global LAST_RESULTS
    u = np.asarray(u, dtype=np.float32)
    W = np.asarray(W, dtype=np.float32)
    nc = _get_nc()
    wr = _pack_wr(W)
    mb = _mask_b()
    md = _mask_d()
    in_maps = []
    for c in range(8):
        u_loc = u[c * B : (c + 1) * B]
        in_maps.append(
            {
                "wr": wr,
                "ut": _pack_ut(u_loc),
                "bdu": _pack_bdu(u_loc),
                "maskb": mb,
                "maskd": md,
            }
        )
    trace = bool(int(os.environ.get("KBENCH_TRACE", "0")))
    try:
        res = run_bass_kernel_spmd(
            nc, in_maps, core_ids=list(range(8)), trace=trace
        )
    except ModuleNotFoundError:
        # axon NTFF hook unavailable in this container; run without trace
        res = run_bass_kernel_spmd(nc, in_maps, core_ids=list(range(8)))
    LAST_RESULTS = res
    outs = [r["v_out"].reshape(B, O, D) for r in res.results]
    return np.concatenate(outs, axis=0).astype(np.float32)


# revision 49
# speedup vs baseline: 2.4288x; 1.0124x over previous
"""CapsuleLayer (dynamic routing) Trainium2 kernel.

Self-contained: shards the full inputs over 8 NeuronCores (data-parallel over
batch), runs a Bass/Tile kernel per core, gathers the full output.

Shapes (full): u [256, 1152, 8] f32, W [1152, 10, 16, 8] f32 -> v [256, 10, 16].
Per core: B=32 batches, W replicated.

Math (per core, ROUTING_ITERS=3):
  u_hat[b,i,od] = sum_k W[i,od,k] * u[b,i,k]          (od = o*16+d)
  b0 = 0; for t in 0..2: c = softmax(b, o); s = sum_i c*u_hat; v = squash(s);
  if t<2: b += sum_d u_hat*v

Device layouts (i = jj*16+g, jj<72, g<16; partitions in [.]):
  Wr  [(g,k)=128, (jj,od)=11520]   f16 (host-pretransposed W)
  uT  [(g,k)=128, (jj,b)=2304]     f16 (host-pretransposed u shard)
  BDu [(g,k)=128, (jj,b8,g')]      f16 block-diag u, host-packed, DMA-streamed
  u_hat [(b8,g16)=128, (jj,od)]    f16, built by PE: BDu.T @ Wr per 8-batch blk
  s matmul: lhsT = block-diag c (f16, built by 4x-mode partition-slice copies
  into a memset-once tile), rhs = u_hat -> psum[(b',o),od] f32
  t=0 needs no per-blk s pass: v0 comes from the dense ut@wr contraction.
  Agreement sum_d u_hat*v: jj 0..49 on DVE (fused scan-MAC + strided diff),
  jj 50..71 on Pool (mul + 4-level tree reduce). Blocks are software-pipelined:
  stage s emits build(s), route(s-2, t2), route(s-1, t1).
"""

import os
import sys

import numpy as np

for _p in ("/opt/trn_rl_repo", "/root/.axon_site/_ro/trn_rl_repo"):
    if os.path.isdir(_p) and _p not in sys.path:
        sys.path.insert(0, _p)

import concourse.bacc as bacc
import concourse.bass as bass
import concourse.mybir as mybir
import concourse.tile as tile

F32 = mybir.dt.float32
F16 = mybir.dt.float16


def _register_scan_mac():
    """Custom DVE op: out[p,k] = cumsum_k(in0*in1) (fp32 state)."""
    import numpy as np

    from concourse import dve_ops as dops
    from concourse.dve_spec import AluOp, Spec, Src0, Src1, lower, scan
    from concourse.dve_uop import DveOpSpec

    name = "SCAN_MAC_ANT"
    for op in dops.OPS:
        if op.name == name:
            return op
    spec = Spec(
        body=scan(AluOp.ADD, Src0 * Src1),
        reference=lambda in0, in1, c0, c1, c2: np.cumsum(
            np.asarray(in0, np.float32).reshape(in0.shape[0], -1)
            * np.asarray(in1, np.float32).reshape(in1.shape[0], -1),
            axis=-1,
        ).reshape(in0.shape),
    )
    shas = {}
    for ver in ("v3", "v4"):
        uops = lower(spec, ver=ver)
        shas[ver] = DveOpSpec(
            name=name, opcode=0, uops=uops, rd1_en=True
        ).sha(ver)
    op = dops.DveOp(name, spec, subdim=False, uops_sha=shas)
    dops.OPS.append(op)
    dops.CUSTOM_DVE_SPECS[name] = spec
    dops._SUB_OPCODE_FOR_NAME[name] = dops._CUSTOM_DVE_ROW_BASE + len(dops.OPS) - 1
    assert dops._SUB_OPCODE_FOR_NAME[name] < 0x20
    return op


_SCAN_MAC = _register_scan_mac()

# Problem constants (per core)
B = 32          # local batch (256 / 8 cores)
I = 1152        # in capsules
O = 10          # out capsules
D = 16          # out dim
K = 8           # in dim
JJ = 72         # i groups of 16
G = 16          # group size
OD = O * D      # 160
BB = 8          # batch block (psum/output partition packing)
NBLK = B // BB  # 4
N_ITERS = 3
NDVE = 2        # scan chunks per (blk,t) on DVE
AC_DEF = 28     # jj per DVE agreement scan chunk
AC_LAST = 30    # the drain route (last blk, t2) is serial: DVE-heavy split
AC_MAX = 30


def _ap(base, free_dims, extra_offset=0):
    """AP with the base's partition dim and explicit free [step, count] dims."""
    return bass.AP(
        tensor=base.tensor,
        offset=base.offset + extra_offset,
        ap=[list(base.ap[0])] + [list(d) for d in free_dims],
    )


def _squash(nc, pool, s_sb, p, v_out):
    """squash over d (16) per o segment. s_sb: [p, 160] f32 sbuf -> v_out."""
    sq = pool.tile([p, OD], F32, tag="sq", bufs=1)
    nc.scalar.square(sq, s_sb)
    nsq = pool.tile([p, O], F32, tag="nsq", bufs=1)
    nc.vector.reduce_sum(
        out=nsq, in_=sq[:].rearrange("p (o d) -> p o d", d=D),
        axis=mybir.AxisListType.X,
    )
    # sqrt(x) = exp(0.5*ln(x)) — keeps ACT on one table set (ln/exp)
    rt = pool.tile([p, O], F32, tag="rt", bufs=1)
    nc.scalar.activation(rt, nsq, mybir.ActivationFunctionType.Ln)
    nc.scalar.activation(rt, rt, mybir.ActivationFunctionType.Exp, scale=0.5)
    nc.vector.tensor_scalar_add(rt, rt, 1e-8)     # + eps
    op1 = pool.tile([p, O], F32, tag="op1", bufs=1)
    nc.vector.tensor_scalar_add(op1, nsq, 1.0)    # 1 + |s|^2
    nc.vector.tensor_mul(op1, op1, rt)            # (1+n)(sqrt+eps)
    rec = pool.tile([p, O], F32, tag="rec", bufs=1)
    nc.vector.reciprocal(rec, op1)
    nc.vector.tensor_mul(rec, rec, nsq)           # n/((1+n)(sqrt+eps))
    nc.vector.tensor_mul(
        v_out[:].rearrange("p (o d) -> p o d", d=D),
        s_sb[:].rearrange("p (o d) -> p o d", d=D),
        _ap(rec[:], [[1, O], [0, D]]),
    )
    return v_out


def _pin_act_table():
    """Make every ACT function we use resolve to the one set containing all
    of them (natural_log_exp_and_others), so bacc hoists a single
    InstLoadActFuncSet instead of thrashing Exp<->Ln sets (~1.3us/load)."""
    from concourse.bacc import get_activation_tables

    tabs = get_activation_tables("gen3")
    keep = "natural_log_exp_and_others"
    if keep not in tabs:
        return
    ours = {
        mybir.ActivationFunctionType.Exp,
        mybir.ActivationFunctionType.Ln,
        mybir.ActivationFunctionType.Square,
        mybir.ActivationFunctionType.Copy,
        mybir.ActivationFunctionType.Identity,
    }
    if not ours <= tabs[keep]:
        return
    for name, s in tabs.items():
        if name != keep:
            s -= ours


def build_program():
    _pin_act_table()
    nc = bacc.Bacc("TRN2")
    wr_d = nc.dram_tensor("wr", [128, JJ * OD], F16, kind="ExternalInput")
    ut_d = nc.dram_tensor("ut", [128, JJ * B], F16, kind="ExternalInput")
    # block-diag u, host-packed contiguous per (blk, ch): [4*8*128, 1152]
    bdu_d = nc.dram_tensor(
        "bdu", [NBLK * 8 * 128, 9 * BB * G], F16, kind="ExternalInput"
    )
    mb_d = nc.dram_tensor("maskb", [128, BB * O], F16, kind="ExternalInput")
    md_d = nc.dram_tensor("maskd", [128, OD], F32, kind="ExternalInput")
    out_d = nc.dram_tensor("v_out", [B, OD], F32, kind="ExternalOutput")

    with tile.TileContext(nc) as tc:
        with (
            tc.tile_pool(name="persist", bufs=1) as persist,
            tc.tile_pool(name="uhat", bufs=4) as uhat_pool,
            tc.tile_pool(name="bdu", bufs=2) as bdu_pool,
            tc.tile_pool(name="pp", bufs=1) as pp_pool,
            tc.tile_pool(name="blog", bufs=2) as blog_pool,
            tc.tile_pool(name="cbuf", bufs=2) as cbuf_pool,
            tc.tile_pool(name="small", bufs=3) as small,
            tc.tile_pool(name="pb", bufs=5, space="PSUM") as pb_pool,
            tc.tile_pool(name="ps", bufs=2, space="PSUM") as ps_pool,
            tc.tile_pool(name="ps0", bufs=1, space="PSUM") as ps0_pool,
        ):
            # ---- resident loads (bulk on the Act HWDGE queue) ----
            ut = persist.tile([128, JJ, B], F16)
            nc.scalar.dma_start(
                out=ut, in_=ut_d[:].rearrange("p (a b) -> p a b", b=B)
            )
            wr = persist.tile([128, JJ, OD], F16)

            def load_wr(ch):
                nc.scalar.dma_start(
                    out=wr[:, ch * 9 : (ch + 1) * 9, :],
                    in_=wr_d[:, ch * 9 * OD : (ch + 1) * 9 * OD].rearrange(
                        "p (a b) -> p a b", b=OD
                    ),
                )

            for _ch in range(8):
                load_wr(_ch)
            maskb = persist.tile([128, BB * O], F16)
            nc.sync.dma_start(out=maskb, in_=mb_d[:])
            maskd = persist.tile([128, OD], F32)
            nc.sync.dma_start(out=maskd, in_=md_d[:])

            # scan scratch: [128, 1+AC*OD] f32, col 0 pinned to zero so the
            # per-segment sums are single strided subs (S[16n+16]-S[16n]).
            scr = persist.tile([128, 1 + AC_MAX * OD], F16)
            nc.vector.memset(scr[:, 0:1], 0.0)
            # block-diag c lhsT tiles, rebuilt by masked multiply per route
            cbd_t = [persist.tile([128, JJ, BB, O], F16, tag=f"cbd{i}",
                                  name=f"cbd{i}")
                     for i in range(2)]

            v0 = persist.tile([B, OD], F32, tag="v0")

            def compute_v0():
                # s0 = 0.1 * sum_i u_hat  (dense (i,k) contraction)
                s0_ps = ps0_pool.tile([B, OD], F32, name="s0_ps")
                for jj in range(JJ):
                    nc.tensor.matmul(
                        s0_ps, lhsT=ut[:, jj, :], rhs=wr[:, jj, :],
                        start=(jj == 0), stop=(jj == JJ - 1),
                    )
                s0_sb = small.tile([B, OD], F32, tag="s0", name="s0_sb")
                nc.scalar.activation(
                    s0_sb, s0_ps, mybir.ActivationFunctionType.Copy, scale=0.1
                )
                _squash(nc, small, s0_sb, B, v0)  # [32, 160]

            state = {}  # blk -> dict(u_hat, blog, blog2, vcur)
            route_n = [0]  # tail emission counter (cbd buffer parity)

            def build(blk):
                u_hat = uhat_pool.tile([128, JJ, OD], F16, name="u_hat")
                state[blk] = {"u_hat": u_hat}
                for ch in range(8):  # 9 jj per chunk
                    bdu = bdu_pool.tile([128, 9, BB, G], F16, name="bdu")
                    nc.sync.dma_start(
                        out=bdu,
                        in_=bdu_d[
                            (blk * 8 + ch) * 128 : (blk * 8 + ch + 1) * 128, :
                        ].rearrange("p (a b g) -> p a b g", b=BB, g=G),
                    )

                    for j3 in range(3):  # 3-jj groups share one psum bank
                        ps = pb_pool.tile([128, 3, OD], F32, name="ps")
                        for j in range(3):
                            jj = ch * 9 + j3 * 3 + j
                            nc.tensor.matmul(
                                ps[:, j, :], lhsT=bdu[:, j3 * 3 + j, :, :],
                                rhs=wr[:, jj, :], start=True, stop=True,
                            )
                        jj0 = ch * 9 + j3 * 3
                        nc.scalar.copy(u_hat[:, jj0 : jj0 + 3, :], ps)

            def route_agree(blk, t):
                st = state[blk]
                u_hat = st["u_hat"]
                # -- vrep[(b,g), od] = v_{t-1}[b, od] --
                vrep = small.tile([128, OD], F32, tag="vrep", name="vrep")
                if t == 1:
                    src = _ap(v0[:], [[0, G], [1, OD]])
                    src = bass.AP(
                        tensor=src.tensor,
                        offset=src.offset + blk * BB * v0[:].ap[0][0],
                        ap=[[v0[:].ap[0][0], BB]] + list(src.ap)[1:],
                    )
                else:
                    vtmp = small.tile([BB, OD], F32, tag="vtmp", name="vtmp",
                                      bufs=4)
                    nc.scalar.dma_start(out=vtmp, in_=st["vcur"])
                    vt = vtmp[:]
                    src = bass.AP(
                        tensor=vt.tensor, offset=vt.offset,
                        ap=[[vt.ap[0][0], BB], [0, G], [1, OD]],
                    )
                nc.scalar.dma_start(out=vrep, in_=src)

                # -- agreement a[b,i,o] = sum_d u_hat*v into bl --
                if t == 1:
                    bl = blog_pool.tile([128, JJ, O], F16, tag="blog",
                                        name="blog", bufs=3)
                    st["blog"] = bl
                else:
                    bl = blog_pool.tile([128, JJ, O], F16, tag="blog2",
                                        name="blog2", bufs=1)
                AC = AC_LAST if (blk >= NBLK - 2 and t == 2) else AC_DEF
                PJJ = JJ - NDVE * AC
                # DVE chunks: fused scan-MAC cumsum, segment sums by
                # strided subtraction against the zero-led scratch.
                for h in range(NDVE):
                    nc.vector._custom_dve(
                        _SCAN_MAC,
                        out=scr[:, 1 : 1 + AC * OD],
                        in0=u_hat[:, h * AC : (h + 1) * AC, :],
                        in1=_ap(vrep[:], [[0, AC], [1, OD]]),
                    )
                    sv = scr[:]
                    nseg = AC * O
                    s_hi = _ap(sv, [[D, nseg]], extra_offset=D)
                    s_lo = _ap(sv, [[D, nseg]], extra_offset=0)
                    blf = bl[:, h * AC : (h + 1) * AC, :].rearrange(
                        "p a o -> p (a o)"
                    )
                    nc.vector.tensor_sub(blf, s_hi, s_lo)
                # Pool chunk: products then 4-level tree reduce over d.
                jp = NDVE * AC
                nsegp = PJJ * O
                pp = pp_pool.tile([128, PJJ * OD], F16, name="pp", bufs=2)
                nc.gpsimd.tensor_mul(
                    pp,
                    u_hat[:, jp:, :].rearrange("p a od -> p (a od)"),
                    _ap(vrep[:], [[0, PJJ], [1, OD]]),
                )
                tr1 = pp_pool.tile([128, nsegp * 8], F16, tag="tr1",
                                   name="tr1", bufs=1)
                nc.gpsimd.tensor_add(
                    tr1[:].rearrange("p (s d) -> p s d", d=8),
                    _ap(pp[:], [[D, nsegp], [1, 8]]),
                    _ap(pp[:], [[D, nsegp], [1, 8]], extra_offset=8),
                )
                tr2 = pp_pool.tile([128, nsegp * 4], F16, tag="tr2",
                                   name="tr2", bufs=1)
                nc.gpsimd.tensor_add(
                    tr2[:].rearrange("p (s d) -> p s d", d=4),
                    _ap(tr1[:], [[8, nsegp], [1, 4]]),
                    _ap(tr1[:], [[8, nsegp], [1, 4]], extra_offset=4),
                )
                tr3 = pp_pool.tile([128, nsegp * 2], F16, tag="tr3",
                                   name="tr3", bufs=1)
                nc.gpsimd.tensor_add(
                    tr3[:].rearrange("p (s d) -> p s d", d=2),
                    _ap(tr2[:], [[4, nsegp], [1, 2]]),
                    _ap(tr2[:], [[4, nsegp], [1, 2]], extra_offset=2),
                )
                blfp = bl[:, jp:, :].rearrange("p a o -> p (a o)")
                nc.gpsimd.tensor_add(
                    blfp,
                    _ap(tr3[:], [[2, nsegp]]),
                    _ap(tr3[:], [[2, nsegp]], extra_offset=1),
                )

                st["bl"] = bl

            def route_tail(blk, t):
                st = state[blk]
                u_hat = st["u_hat"]
                bl = st["bl"]
                # -- c = softmax(blog[+blog2]) over o --
                if t == 1:
                    blogs = bl
                else:
                    blogs = st["blog"]
                    nc.vector.tensor_add(blogs, blogs, bl)
                cb32 = cbuf_pool.tile([128, JJ, O], F32, tag="cb32",
                                      name="cb32")
                nc.scalar.activation(
                    cb32, blogs, mybir.ActivationFunctionType.Exp
                )
                ssum = small.tile([128, JJ], F32, tag="ssum", name="ssum")
                nc.vector.reduce_sum(
                    out=ssum, in_=cb32, axis=mybir.AxisListType.X
                )
                rec = small.tile([128, JJ], F32, tag="srec", name="srec")
                nc.vector.reciprocal(rec, ssum)
                cb2 = cbuf_pool.tile([128, JJ, O], F16, tag="cb2", name="cb2")
                nc.vector.tensor_mul(
                    cb2, cb32, _ap(rec[:], [[1, JJ], [0, O]])
                )

                # -- cbd = cb2 (bcast over b') * maskb (bcast over jj) --
                # all-f16 packed SBUF operands -> DVE 2x mode; two jj-halves
                # so the s-matmul can start on the first half earlier
                cbd = cbd_t[route_n[0] % 2]
                route_n[0] += 1
                JH = JJ // 2
                for half in range(2):
                    nc.vector.tensor_mul(
                        cbd[:, half * JH : (half + 1) * JH, :, :],
                        _ap(cb2[:], [[O, JH], [0, BB], [1, O]],
                            extra_offset=half * JH * O),
                        _ap(maskb[:], [[0, JH], [O, BB], [1, O]]),
                    )

                # -- s matmul: accumulate over jj --
                s_ps = ps_pool.tile([BB * O, OD], F32, name="s_ps")
                for jj in range(JJ):
                    nc.tensor.matmul(
                        s_ps, lhsT=cbd[:, jj, :, :], rhs=u_hat[:, jj, :],
                        start=(jj == 0), stop=(jj == JJ - 1),
                    )

                # -- diag extract: s80[(b,o), d] = s_ps[(b,o), o*16+d] --
                sdm = small.tile([O * BB, OD], F32, tag="sdm", name="sdm")
                nc.vector.tensor_mul(sdm, s_ps, maskd[: O * BB, :])
                s80 = small.tile([O * BB, D], F32, tag="s80", name="s80")
                nc.vector.reduce_sum(
                    out=s80,
                    in_=sdm[:].rearrange("p (o d) -> p d o", d=D),
                    axis=mybir.AxisListType.X,
                )
                # squash on [(b,o), d] with per-partition scalars
                nsq = small.tile([O * BB, 1], F32, tag="nsq80", name="nsq")
                sq = small.tile([O * BB, D], F32, tag="sq80", name="sq")
                nc.scalar.square(sq, s80)
                nc.vector.reduce_sum(
                    out=nsq, in_=sq, axis=mybir.AxisListType.X
                )
                rt = small.tile([O * BB, 1], F32, tag="rt80", name="rt")
                nc.scalar.activation(
                    rt, nsq, mybir.ActivationFunctionType.Ln
                )
                nc.scalar.activation(
                    rt, rt, mybir.ActivationFunctionType.Exp, scale=0.5
                )
                op1 = small.tile([O * BB, 1], F32, tag="op180", name="op1")
                nc.vector.tensor_scalar_add(op1, nsq, 1.0)
                rec80 = small.tile([O * BB, 1], F32, tag="rec80", name="rec80")
                nc.vector.reciprocal(rec80, op1)
                nc.vector.tensor_mul(rec80, rec80, rt)
                vcur = small.tile([O * BB, D], F32, tag="vcur", name="vcur",
                                  bufs=4)
                nc.vector.tensor_scalar_mul(vcur, s80, rec80)
                st["vcur"] = vcur

                if t == N_ITERS - 1:
                    # v_out[blk*8+b, o*16+d] = vcur[b*10+o, d] (same order)
                    nc.scalar.dma_start(
                        out=out_d[blk * BB : (blk + 1) * BB, :], in_=vcur
                    )

            # software pipeline: stage s emits build(s), then both routes'
            # agreement phases (DVE scans + Pool tree) before either tail
            # (softmax/cbd/s-matmul/squash), so ready scans are never stuck
            # behind a tail blocked on the other engine's agreement.
            compute_v0()
            for s in range(NBLK + 1):
                if s < NBLK:
                    build(s)
                if s < NBLK:
                    route_agree(s, 1)
                if s == NBLK:
                    t2s = [NBLK - 2, NBLK - 1]  # drain: both remaining t2s
                elif 2 <= s:
                    t2s = [s - 2]
                else:
                    t2s = []
                for n in t2s:
                    route_agree(n, 2)
                for n in t2s:
                    route_tail(n, 2)
                if s < NBLK:
                    route_tail(s, 1)
    nc.compile()
    return nc


# ---------------- host side ----------------

_NC_CACHE = None


def _get_nc():
    global _NC_CACHE
    if _NC_CACHE is None:
        _NC_CACHE = build_program()
    return _NC_CACHE


def _pack_wr(W):
    # Wr[g*8+k, jj*160 + o*16 + d] = W[jj*16+g, o, d, k]
    return np.ascontiguousarray(
        W.reshape(JJ, G, O, D, K).transpose(1, 4, 0, 2, 3).reshape(128, JJ * OD)
    ).astype(np.float16)


def _pack_ut(u_loc):
    # uT[g*8+k, jj*B + b] = u_loc[b, jj*16+g, k]
    return np.ascontiguousarray(
        u_loc.reshape(B, JJ, G, K).transpose(2, 3, 1, 0).reshape(128, JJ * B)
    ).astype(np.float16)


def _mask_b():
    # maskb[(b,g), (b',o)] = (b' == b), f16
    p = np.arange(128)
    mb = (np.arange(BB)[None, :] == (p // G)[:, None]).astype(np.float16)
    return np.ascontiguousarray(np.repeat(mb, O, axis=1))


def _mask_d():
    # maskd[(b,o) p<80, o'*16+d] = (o' == o); rows >=80 zero
    md = np.zeros((128, OD), dtype=np.float32)
    po = np.arange(O * BB) % O
    for od in range(OD):
        md[: O * BB, od] = (od // D == po).astype(np.float32)
    return md


def _pack_bdu(u_loc):
    # bdu[(blk,ch)*128 + g*8+k, (j, b, g')] = u_loc[blk*8+b, (ch*9+j)*16+g', k]
    #   nonzero only when g' == g; contiguous per (blk, ch) slice.
    u4 = u_loc.reshape(NBLK, BB, JJ // 9, 9, G, K)  # (blk, b, ch, j, g, k)
    out = np.zeros((NBLK, 8, G, K, 9, BB, G), dtype=np.float16)
    for g in range(G):
        # (blk, ch, k, j, b)
        out[:, :, g, :, :, :, g] = u4[:, :, :, :, g, :].transpose(0, 2, 4, 3, 1)
    return np.ascontiguousarray(out.reshape(NBLK * 8 * 128, 9 * BB * G))


LAST_RESULTS = None


def kernel(u, W):
    from concourse.bass_utils import run_bass_kernel_spmd

    global LAST_RESULTS
    u = np.asarray(u, dtype=np.float32)
    W = np.asarray(W, dtype=np.float32)
    nc = _get_nc()
    wr = _pack_wr(W)
    mb = _mask_b()
    md = _mask_d()
    in_maps = []
    for c in range(8):
        u_loc = u[c * B : (c + 1) * B]
        in_maps.append(
            {
                "wr": wr,
                "ut": _pack_ut(u_loc),
                "bdu": _pack_bdu(u_loc),
                "maskb": mb,
                "maskd": md,
            }
        )
    trace = bool(int(os.environ.get("KBENCH_TRACE", "0")))
    try:
        res = run_bass_kernel_spmd(
            nc, in_maps, core_ids=list(range(8)), trace=trace
        )
    except ModuleNotFoundError:
        # axon NTFF hook unavailable in this container; run without trace
        res = run_bass_kernel_spmd(nc, in_maps, core_ids=list(range(8)))
    LAST_RESULTS = res
    outs = [r["v_out"].reshape(B, O, D) for r in res.results]
    return np.concatenate(outs, axis=0).astype(np.float32)


# revision 54
# speedup vs baseline: 2.4337x; 1.0020x over previous
"""CapsuleLayer (dynamic routing) Trainium2 kernel.

Self-contained: shards the full inputs over 8 NeuronCores (data-parallel over
batch), runs a Bass/Tile kernel per core, gathers the full output.

Shapes (full): u [256, 1152, 8] f32, W [1152, 10, 16, 8] f32 -> v [256, 10, 16].
Per core: B=32 batches, W replicated.

Math (per core, ROUTING_ITERS=3):
  u_hat[b,i,od] = sum_k W[i,od,k] * u[b,i,k]          (od = o*16+d)
  b0 = 0; for t in 0..2: c = softmax(b, o); s = sum_i c*u_hat; v = squash(s);
  if t<2: b += sum_d u_hat*v

Device layouts (i = jj*16+g, jj<72, g<16; partitions in [.]):
  Wr  [(g,k)=128, (jj,od)=11520]   f16 (host-pretransposed W)
  uT  [(g,k)=128, (jj,b)=2304]     f16 (host-pretransposed u shard)
  BDu [(g,k)=128, (jj,b8,g')]      f16 block-diag u, host-packed, DMA-streamed
  u_hat [(b8,g16)=128, (jj,od)]    f16, built by PE: BDu.T @ Wr per 8-batch blk
  s matmul: lhsT = block-diag c (f16, built by 4x-mode partition-slice copies
  into a memset-once tile), rhs = u_hat -> psum[(b',o),od] f32
  t=0 needs no per-blk s pass: v0 comes from the dense ut@wr contraction.
  Agreement sum_d u_hat*v: jj 0..49 on DVE (fused scan-MAC + strided diff),
  jj 50..71 on Pool (mul + 4-level tree reduce). Blocks are software-pipelined:
  stage s emits build(s), route(s-2, t2), route(s-1, t1).
"""

import os
import sys

import numpy as np

for _p in ("/opt/trn_rl_repo", "/root/.axon_site/_ro/trn_rl_repo"):
    if os.path.isdir(_p) and _p not in sys.path:
        sys.path.insert(0, _p)

import concourse.bacc as bacc
import concourse.bass as bass
import concourse.mybir as mybir
import concourse.tile as tile

F32 = mybir.dt.float32
F16 = mybir.dt.float16


def _register_scan_mac():
    """Custom DVE op: out[p,k] = cumsum_k(in0*in1) (fp32 state)."""
    import numpy as np

    from concourse import dve_ops as dops
    from concourse.dve_spec import AluOp, Spec, Src0, Src1, lower, scan
    from concourse.dve_uop import DveOpSpec

    name = "SCAN_MAC_ANT"
    for op in dops.OPS:
        if op.name == name:
            return op
    spec = Spec(
        body=scan(AluOp.ADD, Src0 * Src1),
        reference=lambda in0, in1, c0, c1, c2: np.cumsum(
            np.asarray(in0, np.float32).reshape(in0.shape[0], -1)
            * np.asarray(in1, np.float32).reshape(in1.shape[0], -1),
            axis=-1,
        ).reshape(in0.shape),
    )
    shas = {}
    for ver in ("v3", "v4"):
        uops = lower(spec, ver=ver)
        shas[ver] = DveOpSpec(
            name=name, opcode=0, uops=uops, rd1_en=True
        ).sha(ver)
    op = dops.DveOp(name, spec, subdim=False, uops_sha=shas)
    dops.OPS.append(op)
    dops.CUSTOM_DVE_SPECS[name] = spec
    dops._SUB_OPCODE_FOR_NAME[name] = dops._CUSTOM_DVE_ROW_BASE + len(dops.OPS) - 1
    assert dops._SUB_OPCODE_FOR_NAME[name] < 0x20
    return op


_SCAN_MAC = _register_scan_mac()

# Problem constants (per core)
B = 32          # local batch (256 / 8 cores)
I = 1152        # in capsules
O = 10          # out capsules
D = 16          # out dim
K = 8           # in dim
JJ = 72         # i groups of 16
G = 16          # group size
OD = O * D      # 160
BB = 8          # batch block (psum/output partition packing)
NBLK = B // BB  # 4
N_ITERS = 3
NDVE = 2        # scan chunks per (blk,t) on DVE
AC_DEF = 28     # jj per DVE agreement scan chunk
AC_LAST = 30    # the drain route (last blk, t2) is serial: DVE-heavy split
AC_MAX = 30


def _ap(base, free_dims, extra_offset=0):
    """AP with the base's partition dim and explicit free [step, count] dims."""
    return bass.AP(
        tensor=base.tensor,
        offset=base.offset + extra_offset,
        ap=[list(base.ap[0])] + [list(d) for d in free_dims],
    )


def _squash(nc, pool, s_sb, p, v_out):
    """squash over d (16) per o segment. s_sb: [p, 160] f32 sbuf -> v_out."""
    sq = pool.tile([p, OD], F32, tag="sq", bufs=1)
    nc.scalar.square(sq, s_sb)
    nsq = pool.tile([p, O], F32, tag="nsq", bufs=1)
    nc.vector.reduce_sum(
        out=nsq, in_=sq[:].rearrange("p (o d) -> p o d", d=D),
        axis=mybir.AxisListType.X,
    )
    # sqrt(x) = exp(0.5*ln(x)) — keeps ACT on one table set (ln/exp)
    rt = pool.tile([p, O], F32, tag="rt", bufs=1)
    nc.scalar.activation(rt, nsq, mybir.ActivationFunctionType.Ln)
    nc.scalar.activation(rt, rt, mybir.ActivationFunctionType.Exp, scale=0.5)
    nc.vector.tensor_scalar_add(rt, rt, 1e-8)     # + eps
    op1 = pool.tile([p, O], F32, tag="op1", bufs=1)
    nc.vector.tensor_scalar_add(op1, nsq, 1.0)    # 1 + |s|^2
    nc.vector.tensor_mul(op1, op1, rt)            # (1+n)(sqrt+eps)
    rec = pool.tile([p, O], F32, tag="rec", bufs=1)
    nc.vector.reciprocal(rec, op1)
    nc.vector.tensor_mul(rec, rec, nsq)           # n/((1+n)(sqrt+eps))
    nc.vector.tensor_mul(
        v_out[:].rearrange("p (o d) -> p o d", d=D),
        s_sb[:].rearrange("p (o d) -> p o d", d=D),
        _ap(rec[:], [[1, O], [0, D]]),
    )
    return v_out


def _pin_act_table():
    """Make every ACT function we use resolve to the one set containing all
    of them (natural_log_exp_and_others), so bacc hoists a single
    InstLoadActFuncSet instead of thrashing Exp<->Ln sets (~1.3us/load)."""
    from concourse.bacc import get_activation_tables

    tabs = get_activation_tables("gen3")
    keep = "natural_log_exp_and_others"
    if keep not in tabs:
        return
    ours = {
        mybir.ActivationFunctionType.Exp,
        mybir.ActivationFunctionType.Ln,
        mybir.ActivationFunctionType.Square,
        mybir.ActivationFunctionType.Copy,
        mybir.ActivationFunctionType.Identity,
    }
    if not ours <= tabs[keep]:
        return
    for name, s in tabs.items():
        if name != keep:
            s -= ours


def build_program():
    _pin_act_table()
    nc = bacc.Bacc("TRN2")
    wr_d = nc.dram_tensor("wr", [128, JJ * OD], F16, kind="ExternalInput")
    ut_d = nc.dram_tensor("ut", [128, JJ * B], F16, kind="ExternalInput")
    # block-diag u, host-packed contiguous per (blk, ch): [4*8*128, 1152]
    bdu_d = nc.dram_tensor(
        "bdu", [NBLK * 8 * 128, 9 * BB * G], F16, kind="ExternalInput"
    )
    mb_d = nc.dram_tensor("maskb", [128, BB * O], F16, kind="ExternalInput")
    md_d = nc.dram_tensor("maskd", [128, OD], F32, kind="ExternalInput")
    out_d = nc.dram_tensor("v_out", [B, OD], F32, kind="ExternalOutput")

    with tile.TileContext(nc) as tc:
        with (
            tc.tile_pool(name="persist", bufs=1) as persist,
            tc.tile_pool(name="uhat", bufs=4) as uhat_pool,
            tc.tile_pool(name="bdu", bufs=2) as bdu_pool,
            tc.tile_pool(name="pp", bufs=1) as pp_pool,
            tc.tile_pool(name="blog", bufs=2) as blog_pool,
            tc.tile_pool(name="cbuf", bufs=2) as cbuf_pool,
            tc.tile_pool(name="small", bufs=3) as small,
            tc.tile_pool(name="pb", bufs=4, space="PSUM") as pb_pool,
            tc.tile_pool(name="ps", bufs=3, space="PSUM") as ps_pool,
            tc.tile_pool(name="ps0", bufs=1, space="PSUM") as ps0_pool,
        ):
            # ---- resident loads (bulk on the Act HWDGE queue) ----
            ut = persist.tile([128, JJ, B], F16)
            nc.scalar.dma_start(
                out=ut, in_=ut_d[:].rearrange("p (a b) -> p a b", b=B)
            )
            wr = persist.tile([128, JJ, OD], F16)

            def load_wr(ch):
                nc.scalar.dma_start(
                    out=wr[:, ch * 9 : (ch + 1) * 9, :],
                    in_=wr_d[:, ch * 9 * OD : (ch + 1) * 9 * OD].rearrange(
                        "p (a b) -> p a b", b=OD
                    ),
                )

            for _ch in range(8):
                load_wr(_ch)
            maskb = persist.tile([128, BB * O], F16)
            nc.sync.dma_start(out=maskb, in_=mb_d[:])
            maskd = persist.tile([128, OD], F32)
            nc.sync.dma_start(out=maskd, in_=md_d[:])

            # scan scratch: [128, 1+AC*OD] f32, col 0 pinned to zero so the
            # per-segment sums are single strided subs (S[16n+16]-S[16n]).
            scr = persist.tile([128, 1 + AC_MAX * OD], F16)
            nc.vector.memset(scr[:, 0:1], 0.0)
            # block-diag c lhsT tiles, rebuilt by masked multiply per route
            cbd_t = [persist.tile([128, JJ, BB, O], F16, tag=f"cbd{i}",
                                  name=f"cbd{i}")
                     for i in range(2)]

            v0 = persist.tile([B, OD], F32, tag="v0")

            def compute_v0():
                # s0 = 0.1 * sum_i u_hat  (dense (i,k) contraction)
                s0_ps = ps0_pool.tile([B, OD], F32, name="s0_ps")
                for jj in range(JJ):
                    nc.tensor.matmul(
                        s0_ps, lhsT=ut[:, jj, :], rhs=wr[:, jj, :],
                        start=(jj == 0), stop=(jj == JJ - 1),
                    )
                s0_sb = small.tile([B, OD], F32, tag="s0", name="s0_sb")
                nc.scalar.activation(
                    s0_sb, s0_ps, mybir.ActivationFunctionType.Copy, scale=0.1
                )
                _squash(nc, small, s0_sb, B, v0)  # [32, 160]

            state = {}  # blk -> dict(u_hat, blog, blog2, vcur)
            route_n = [0]  # tail emission counter (cbd buffer parity)

            def build(blk):
                u_hat = uhat_pool.tile([128, JJ, OD], F16, name="u_hat")
                state[blk] = {"u_hat": u_hat}
                for ch in range(8):  # 9 jj per chunk
                    bdu = bdu_pool.tile([128, 9, BB, G], F16, name="bdu")
                    nc.sync.dma_start(
                        out=bdu,
                        in_=bdu_d[
                            (blk * 8 + ch) * 128 : (blk * 8 + ch + 1) * 128, :
                        ].rearrange("p (a b g) -> p a b g", b=BB, g=G),
                    )

                    for j3 in range(3):  # 3-jj groups share one psum bank
                        ps = pb_pool.tile([128, 3, OD], F32, name="ps")
                        for j in range(3):
                            jj = ch * 9 + j3 * 3 + j
                            nc.tensor.matmul(
                                ps[:, j, :], lhsT=bdu[:, j3 * 3 + j, :, :],
                                rhs=wr[:, jj, :], start=True, stop=True,
                            )
                        jj0 = ch * 9 + j3 * 3
                        nc.scalar.copy(u_hat[:, jj0 : jj0 + 3, :], ps)

            def route_agree(blk, t):
                st = state[blk]
                u_hat = st["u_hat"]
                # -- vrep[(b,g), od] = v_{t-1}[b, od] --
                vrep = small.tile([128, OD], F32, tag="vrep", name="vrep")
                if t == 1:
                    src = _ap(v0[:], [[0, G], [1, OD]])
                    src = bass.AP(
                        tensor=src.tensor,
                        offset=src.offset + blk * BB * v0[:].ap[0][0],
                        ap=[[v0[:].ap[0][0], BB]] + list(src.ap)[1:],
                    )
                else:
                    vtmp = small.tile([BB, OD], F32, tag="vtmp", name="vtmp",
                                      bufs=4)
                    nc.scalar.dma_start(out=vtmp, in_=st["vcur"])
                    vt = vtmp[:]
                    src = bass.AP(
                        tensor=vt.tensor, offset=vt.offset,
                        ap=[[vt.ap[0][0], BB], [0, G], [1, OD]],
                    )
                nc.scalar.dma_start(out=vrep, in_=src)

                # -- agreement a[b,i,o] = sum_d u_hat*v into bl --
                if t == 1:
                    bl = blog_pool.tile([128, JJ, O], F16, tag="blog",
                                        name="blog", bufs=3)
                    st["blog"] = bl
                else:
                    bl = blog_pool.tile([128, JJ, O], F16, tag="blog2",
                                        name="blog2", bufs=1)
                AC = AC_LAST if (blk >= NBLK - 2 and t == 2) else AC_DEF
                PJJ = JJ - NDVE * AC
                # DVE chunks: fused scan-MAC cumsum, segment sums by
                # strided subtraction against the zero-led scratch.
                for h in range(NDVE):
                    nc.vector._custom_dve(
                        _SCAN_MAC,
                        out=scr[:, 1 : 1 + AC * OD],
                        in0=u_hat[:, h * AC : (h + 1) * AC, :],
                        in1=_ap(vrep[:], [[0, AC], [1, OD]]),
                    )
                    sv = scr[:]
                    nseg = AC * O
                    s_hi = _ap(sv, [[D, nseg]], extra_offset=D)
                    s_lo = _ap(sv, [[D, nseg]], extra_offset=0)
                    blf = bl[:, h * AC : (h + 1) * AC, :].rearrange(
                        "p a o -> p (a o)"
                    )
                    nc.vector.tensor_sub(blf, s_hi, s_lo)
                # Pool chunk: products then 4-level tree reduce over d.
                jp = NDVE * AC
                nsegp = PJJ * O
                pp = pp_pool.tile([128, PJJ * OD], F16, name="pp", bufs=2)
                nc.gpsimd.tensor_mul(
                    pp,
                    u_hat[:, jp:, :].rearrange("p a od -> p (a od)"),
                    _ap(vrep[:], [[0, PJJ], [1, OD]]),
                )
                tr1 = pp_pool.tile([128, nsegp * 8], F16, tag="tr1",
                                   name="tr1", bufs=1)
                nc.gpsimd.tensor_add(
                    tr1[:].rearrange("p (s d) -> p s d", d=8),
                    _ap(pp[:], [[D, nsegp], [1, 8]]),
                    _ap(pp[:], [[D, nsegp], [1, 8]], extra_offset=8),
                )
                tr2 = pp_pool.tile([128, nsegp * 4], F16, tag="tr2",
                                   name="tr2", bufs=1)
                nc.gpsimd.tensor_add(
                    tr2[:].rearrange("p (s d) -> p s d", d=4),
                    _ap(tr1[:], [[8, nsegp], [1, 4]]),
                    _ap(tr1[:], [[8, nsegp], [1, 4]], extra_offset=4),
                )
                tr3 = pp_pool.tile([128, nsegp * 2], F16, tag="tr3",
                                   name="tr3", bufs=1)
                nc.gpsimd.tensor_add(
                    tr3[:].rearrange("p (s d) -> p s d", d=2),
                    _ap(tr2[:], [[4, nsegp], [1, 2]]),
                    _ap(tr2[:], [[4, nsegp], [1, 2]], extra_offset=2),
                )
                blfp = bl[:, jp:, :].rearrange("p a o -> p (a o)")
                nc.gpsimd.tensor_add(
                    blfp,
                    _ap(tr3[:], [[2, nsegp]]),
                    _ap(tr3[:], [[2, nsegp]], extra_offset=1),
                )

                st["bl"] = bl

            def route_tail(blk, t):
                st = state[blk]
                u_hat = st["u_hat"]
                bl = st["bl"]
                # -- c = softmax(blog[+blog2]) over o --
                if t == 1:
                    blogs = bl
                else:
                    blogs = st["blog"]
                    nc.vector.tensor_add(blogs, blogs, bl)
                cb32 = cbuf_pool.tile([128, JJ, O], F32, tag="cb32",
                                      name="cb32")
                nc.scalar.activation(
                    cb32, blogs, mybir.ActivationFunctionType.Exp
                )
                ssum = small.tile([128, JJ], F32, tag="ssum", name="ssum")
                nc.vector.reduce_sum(
                    out=ssum, in_=cb32, axis=mybir.AxisListType.X
                )
                rec = small.tile([128, JJ], F32, tag="srec", name="srec")
                nc.vector.reciprocal(rec, ssum)
                cb2 = cbuf_pool.tile([128, JJ, O], F16, tag="cb2", name="cb2")
                nc.vector.tensor_mul(
                    cb2, cb32, _ap(rec[:], [[1, JJ], [0, O]])
                )

                # -- cbd = cb2 (bcast over b') * maskb (bcast over jj) --
                # all-f16 packed SBUF operands -> DVE 2x mode; two jj-halves
                # so the s-matmul can start on the first half earlier
                cbd = cbd_t[route_n[0] % 2]
                route_n[0] += 1
                JH = JJ // 2
                for half in range(2):
                    nc.vector.tensor_mul(
                        cbd[:, half * JH : (half + 1) * JH, :, :],
                        _ap(cb2[:], [[O, JH], [0, BB], [1, O]],
                            extra_offset=half * JH * O),
                        _ap(maskb[:], [[0, JH], [O, BB], [1, O]]),
                    )

                # -- s matmul: accumulate over jj --
                s_ps = ps_pool.tile([BB * O, OD], F32, name="s_ps")
                for jj in range(JJ):
                    nc.tensor.matmul(
                        s_ps, lhsT=cbd[:, jj, :, :], rhs=u_hat[:, jj, :],
                        start=(jj == 0), stop=(jj == JJ - 1),
                    )

                # -- diag extract: s80[(b,o), d] = s_ps[(b,o), o*16+d] --
                sdm = small.tile([O * BB, OD], F32, tag="sdm", name="sdm")
                nc.vector.tensor_mul(sdm, s_ps, maskd[: O * BB, :])
                s80 = small.tile([O * BB, D], F32, tag="s80", name="s80")
                nc.vector.reduce_sum(
                    out=s80,
                    in_=sdm[:].rearrange("p (o d) -> p d o", d=D),
                    axis=mybir.AxisListType.X,
                )
                # squash on [(b,o), d] with per-partition scalars
                nsq = small.tile([O * BB, 1], F32, tag="nsq80", name="nsq")
                sq = small.tile([O * BB, D], F32, tag="sq80", name="sq")
                nc.scalar.square(sq, s80)
                nc.vector.reduce_sum(
                    out=nsq, in_=sq, axis=mybir.AxisListType.X
                )
                rt = small.tile([O * BB, 1], F32, tag="rt80", name="rt")
                nc.scalar.activation(
                    rt, nsq, mybir.ActivationFunctionType.Ln
                )
                nc.scalar.activation(
                    rt, rt, mybir.ActivationFunctionType.Exp, scale=0.5
                )
                op1 = small.tile([O * BB, 1], F32, tag="op180", name="op1")
                nc.vector.tensor_scalar_add(op1, nsq, 1.0)
                rec80 = small.tile([O * BB, 1], F32, tag="rec80", name="rec80")
                nc.vector.reciprocal(rec80, op1)
                nc.vector.tensor_mul(rec80, rec80, rt)
                vcur = small.tile([O * BB, D], F32, tag="vcur", name="vcur",
                                  bufs=4)
                nc.vector.tensor_scalar_mul(vcur, s80, rec80)
                st["vcur"] = vcur

                if t == N_ITERS - 1:
                    # v_out[blk*8+b, o*16+d] = vcur[b*10+o, d] (same order)
                    nc.scalar.dma_start(
                        out=out_d[blk * BB : (blk + 1) * BB, :], in_=vcur
                    )

            # software pipeline: stage s emits build(s), then both routes'
            # agreement phases (DVE scans + Pool tree) before either tail
            # (softmax/cbd/s-matmul/squash), so ready scans are never stuck
            # behind a tail blocked on the other engine's agreement.
            compute_v0()
            for s in range(NBLK + 1):
                if s < NBLK:
                    build(s)
                if s < NBLK:
                    route_agree(s, 1)
                if s == NBLK:
                    t2s = [NBLK - 2, NBLK - 1]  # drain: both remaining t2s
                elif 2 <= s:
                    t2s = [s - 2]
                else:
                    t2s = []
                for n in t2s:
                    route_agree(n, 2)
                for n in t2s:
                    route_tail(n, 2)
                if s < NBLK:
                    route_tail(s, 1)
    nc.compile()
    return nc


# ---------------- host side ----------------

_NC_CACHE = None


def _get_nc():
    global _NC_CACHE
    if _NC_CACHE is None:
        _NC_CACHE = build_program()
    return _NC_CACHE


def _pack_wr(W):
    # Wr[g*8+k, jj*160 + o*16 + d] = W[jj*16+g, o, d, k]
    return np.ascontiguousarray(
        W.reshape(JJ, G, O, D, K).transpose(1, 4, 0, 2, 3).reshape(128, JJ * OD)
    ).astype(np.float16)


def _pack_ut(u_loc):
    # uT[g*8+k, jj*B + b] = u_loc[b, jj*16+g, k]
    return np.ascontiguousarray(
        u_loc.reshape(B, JJ, G, K).transpose(2, 3, 1, 0).reshape(128, JJ * B)
    ).astype(np.float16)


def _mask_b():
    # maskb[(b,g), (b',o)] = (b' == b), f16
    p = np.arange(128)
    mb = (np.arange(BB)[None, :] == (p // G)[:, None]).astype(np.float16)
    return np.ascontiguousarray(np.repeat(mb, O, axis=1))


def _mask_d():
    # maskd[(b,o) p<80, o'*16+d] = (o' == o); rows >=80 zero
    md = np.zeros((128, OD), dtype=np.float32)
    po = np.arange(O * BB) % O
    for od in range(OD):
        md[: O * BB, od] = (od // D == po).astype(np.float32)
    return md


def _pack_bdu(u_loc):
    # bdu[(blk,ch)*128 + g*8+k, (j, b, g')] = u_loc[blk*8+b, (ch*9+j)*16+g', k]
    #   nonzero only when g' == g; contiguous per (blk, ch) slice.
    u4 = u_loc.reshape(NBLK, BB, JJ // 9, 9, G, K)  # (blk, b, ch, j, g, k)
    out = np.zeros((NBLK, 8, G, K, 9, BB, G), dtype=np.float16)
    for g in range(G):
        # (blk, ch, k, j, b)
        out[:, :, g, :, :, :, g] = u4[:, :, :, :, g, :].transpose(0, 2, 4, 3, 1)
    return np.ascontiguousarray(out.reshape(NBLK * 8 * 128, 9 * BB * G))


LAST_RESULTS = None


def kernel(u, W):
    from concourse.bass_utils import run_bass_kernel_spmd

    global LAST_RESULTS
    u = np.asarray(u, dtype=np.float32)
    W = np.asarray(W, dtype=np.float32)
    nc = _get_nc()
    wr = _pack_wr(W)
    mb = _mask_b()
    md = _mask_d()
    in_maps = []
    for c in range(8):
        u_loc = u[c * B : (c + 1) * B]
        in_maps.append(
            {
                "wr": wr,
                "ut": _pack_ut(u_loc),
                "bdu": _pack_bdu(u_loc),
                "maskb": mb,
                "maskd": md,
            }
        )
    trace = bool(int(os.environ.get("KBENCH_TRACE", "0")))
    try:
        res = run_bass_kernel_spmd(
            nc, in_maps, core_ids=list(range(8)), trace=trace
        )
    except ModuleNotFoundError:
        # axon NTFF hook unavailable in this container; run without trace
        res = run_bass_kernel_spmd(nc, in_maps, core_ids=list(range(8)))
    LAST_RESULTS = res
    outs = [r["v_out"].reshape(B, O, D) for r in res.results]
    return np.concatenate(outs, axis=0).astype(np.float32)


# revision 59
# speedup vs baseline: 2.4483x; 1.0060x over previous
"""CapsuleLayer (dynamic routing) Trainium2 kernel.

Self-contained: shards the full inputs over 8 NeuronCores (data-parallel over
batch), runs a Bass/Tile kernel per core, gathers the full output.

Shapes (full): u [256, 1152, 8] f32, W [1152, 10, 16, 8] f32 -> v [256, 10, 16].
Per core: B=32 batches, W replicated.

Math (per core, ROUTING_ITERS=3):
  u_hat[b,i,od] = sum_k W[i,od,k] * u[b,i,k]          (od = o*16+d)
  b0 = 0; for t in 0..2: c = softmax(b, o); s = sum_i c*u_hat; v = squash(s);
  if t<2: b += sum_d u_hat*v

Device layouts (i = jj*16+g, jj<72, g<16; partitions in [.]):
  Wr  [(g,k)=128, (jj,od)=11520]   f16 (host-pretransposed W)
  uT  [(g,k)=128, (jj,b)=2304]     f16 (host-pretransposed u shard)
  BDu [(g,k)=128, (jj,b8,g')]      f16 block-diag u, host-packed, DMA-streamed
  u_hat [(b8,g16)=128, (jj,od)]    f16, built by PE: BDu.T @ Wr per 8-batch blk
  s matmul: lhsT = block-diag c (f16, built by 4x-mode partition-slice copies
  into a memset-once tile), rhs = u_hat -> psum[(b',o),od] f32
  t=0 needs no per-blk s pass: v0 comes from the dense ut@wr contraction.
  Agreement sum_d u_hat*v: jj 0..49 on DVE (fused scan-MAC + strided diff),
  jj 50..71 on Pool (mul + 4-level tree reduce). Blocks are software-pipelined:
  stage s emits build(s), route(s-2, t2), route(s-1, t1).
"""

import os
import sys

import numpy as np

for _p in ("/opt/trn_rl_repo", "/root/.axon_site/_ro/trn_rl_repo"):
    if os.path.isdir(_p) and _p not in sys.path:
        sys.path.insert(0, _p)

import concourse.bacc as bacc
import concourse.bass as bass
import concourse.mybir as mybir
import concourse.tile as tile

F32 = mybir.dt.float32
F16 = mybir.dt.float16


def _register_scan_mac():
    """Custom DVE op: out[p,k] = cumsum_k(in0*in1) (fp32 state)."""
    import numpy as np

    from concourse import dve_ops as dops
    from concourse.dve_spec import AluOp, Spec, Src0, Src1, lower, scan
    from concourse.dve_uop import DveOpSpec

    name = "SCAN_MAC_ANT"
    for op in dops.OPS:
        if op.name == name:
            return op
    spec = Spec(
        body=scan(AluOp.ADD, Src0 * Src1),
        reference=lambda in0, in1, c0, c1, c2: np.cumsum(
            np.asarray(in0, np.float32).reshape(in0.shape[0], -1)
            * np.asarray(in1, np.float32).reshape(in1.shape[0], -1),
            axis=-1,
        ).reshape(in0.shape),
    )
    shas = {}
    for ver in ("v3", "v4"):
        uops = lower(spec, ver=ver)
        shas[ver] = DveOpSpec(
            name=name, opcode=0, uops=uops, rd1_en=True
        ).sha(ver)
    op = dops.DveOp(name, spec, subdim=False, uops_sha=shas)
    dops.OPS.append(op)
    dops.CUSTOM_DVE_SPECS[name] = spec
    dops._SUB_OPCODE_FOR_NAME[name] = dops._CUSTOM_DVE_ROW_BASE + len(dops.OPS) - 1
    assert dops._SUB_OPCODE_FOR_NAME[name] < 0x20
    return op


_SCAN_MAC = _register_scan_mac()

# Problem constants (per core)
B = 32          # local batch (256 / 8 cores)
I = 1152        # in capsules
O = 10          # out capsules
D = 16          # out dim
K = 8           # in dim
JJ = 72         # i groups of 16
G = 16          # group size
OD = O * D      # 160
BB = 8          # batch block (psum/output partition packing)
NBLK = B // BB  # 4
N_ITERS = 3
NDVE = 2        # scan chunks per (blk,t) on DVE
AC_DEF = 28     # jj per DVE agreement scan chunk
AC_LAST = 28    # the drain route (last blk, t2) is serial: DVE-heavy split
AC_MAX = 28


def _ap(base, free_dims, extra_offset=0):
    """AP with the base's partition dim and explicit free [step, count] dims."""
    return bass.AP(
        tensor=base.tensor,
        offset=base.offset + extra_offset,
        ap=[list(base.ap[0])] + [list(d) for d in free_dims],
    )


def _squash(nc, pool, s_sb, p, v_out):
    """squash over d (16) per o segment. s_sb: [p, 160] f32 sbuf -> v_out."""
    sq = pool.tile([p, OD], F32, tag="sq", bufs=1)
    nc.scalar.square(sq, s_sb)
    nsq = pool.tile([p, O], F32, tag="nsq", bufs=1)
    nc.vector.reduce_sum(
        out=nsq, in_=sq[:].rearrange("p (o d) -> p o d", d=D),
        axis=mybir.AxisListType.X,
    )
    # sqrt(x) = exp(0.5*ln(x)) — keeps ACT on one table set (ln/exp)
    rt = pool.tile([p, O], F32, tag="rt", bufs=1)
    nc.scalar.activation(rt, nsq, mybir.ActivationFunctionType.Ln)
    nc.scalar.activation(rt, rt, mybir.ActivationFunctionType.Exp, scale=0.5)
    nc.vector.tensor_scalar_add(rt, rt, 1e-8)     # + eps
    op1 = pool.tile([p, O], F32, tag="op1", bufs=1)
    nc.vector.tensor_scalar_add(op1, nsq, 1.0)    # 1 + |s|^2
    nc.vector.tensor_mul(op1, op1, rt)            # (1+n)(sqrt+eps)
    rec = pool.tile([p, O], F32, tag="rec", bufs=1)
    nc.vector.reciprocal(rec, op1)
    nc.vector.tensor_mul(rec, rec, nsq)           # n/((1+n)(sqrt+eps))
    nc.vector.tensor_mul(
        v_out[:].rearrange("p (o d) -> p o d", d=D),
        s_sb[:].rearrange("p (o d) -> p o d", d=D),
        _ap(rec[:], [[1, O], [0, D]]),
    )
    return v_out


def _pin_act_table():
    """Make every ACT function we use resolve to the one set containing all
    of them (natural_log_exp_and_others), so bacc hoists a single
    InstLoadActFuncSet instead of thrashing Exp<->Ln sets (~1.3us/load)."""
    from concourse.bacc import get_activation_tables

    tabs = get_activation_tables("gen3")
    keep = "natural_log_exp_and_others"
    if keep not in tabs:
        return
    ours = {
        mybir.ActivationFunctionType.Exp,
        mybir.ActivationFunctionType.Ln,
        mybir.ActivationFunctionType.Square,
        mybir.ActivationFunctionType.Copy,
        mybir.ActivationFunctionType.Identity,
    }
    if not ours <= tabs[keep]:
        return
    for name, s in tabs.items():
        if name != keep:
            s -= ours


def build_program():
    _pin_act_table()
    nc = bacc.Bacc("TRN2")
    wr_d = nc.dram_tensor("wr", [128, JJ * OD], F16, kind="ExternalInput")
    ut_d = nc.dram_tensor("ut", [128, JJ * B], F16, kind="ExternalInput")
    # block-diag u, host-packed contiguous per (blk, ch): [4*8*128, 1152]
    bdu_d = nc.dram_tensor(
        "bdu", [NBLK * 8 * 128, 9 * BB * G], F16, kind="ExternalInput"
    )
    mb_d = nc.dram_tensor("maskb", [128, BB * O], F16, kind="ExternalInput")
    md_d = nc.dram_tensor("maskd", [128, OD], F32, kind="ExternalInput")
    out_d = nc.dram_tensor("v_out", [B, OD], F32, kind="ExternalOutput")

    with tile.TileContext(nc) as tc:
        with (
            tc.tile_pool(name="persist", bufs=1) as persist,
            tc.tile_pool(name="uhat", bufs=4) as uhat_pool,
            tc.tile_pool(name="bdu", bufs=2) as bdu_pool,
            tc.tile_pool(name="pp", bufs=1) as pp_pool,
            tc.tile_pool(name="blog", bufs=2) as blog_pool,
            tc.tile_pool(name="cbuf", bufs=2) as cbuf_pool,
            tc.tile_pool(name="small", bufs=3) as small,
            tc.tile_pool(name="pb", bufs=4, space="PSUM") as pb_pool,
            tc.tile_pool(name="ps", bufs=3, space="PSUM") as ps_pool,
            tc.tile_pool(name="ps0", bufs=1, space="PSUM") as ps0_pool,
        ):
            # ---- resident loads (bulk on the Act HWDGE queue) ----
            ut = persist.tile([128, JJ, B], F16)
            nc.scalar.dma_start(
                out=ut, in_=ut_d[:].rearrange("p (a b) -> p a b", b=B)
            )
            wr = persist.tile([128, JJ, OD], F16)

            def load_wr(ch):
                nc.scalar.dma_start(
                    out=wr[:, ch * 9 : (ch + 1) * 9, :],
                    in_=wr_d[:, ch * 9 * OD : (ch + 1) * 9 * OD].rearrange(
                        "p (a b) -> p a b", b=OD
                    ),
                )

            for _ch in range(8):
                load_wr(_ch)
            maskb = persist.tile([128, BB * O], F16)
            nc.sync.dma_start(out=maskb, in_=mb_d[:])
            maskd = persist.tile([128, OD], F32)
            nc.sync.dma_start(out=maskd, in_=md_d[:])

            # scan scratch: [128, 1+AC*OD] f32, col 0 pinned to zero so the
            # per-segment sums are single strided subs (S[16n+16]-S[16n]).
            scr = persist.tile([128, 1 + AC_MAX * OD], F16)
            nc.vector.memset(scr[:, 0:1], 0.0)
            # block-diag c lhsT tiles, rebuilt by masked multiply per route
            cbd_t = [persist.tile([128, JJ, BB, O], F16, tag=f"cbd{i}",
                                  name=f"cbd{i}")
                     for i in range(2)]

            v0 = persist.tile([B, OD], F32, tag="v0")

            def compute_v0():
                # s0 = 0.1 * sum_i u_hat  (dense (i,k) contraction)
                s0_ps = ps0_pool.tile([B, OD], F32, name="s0_ps")
                for jj in range(JJ):
                    nc.tensor.matmul(
                        s0_ps, lhsT=ut[:, jj, :], rhs=wr[:, jj, :],
                        start=(jj == 0), stop=(jj == JJ - 1),
                    )
                s0_sb = small.tile([B, OD], F32, tag="s0", name="s0_sb")
                nc.scalar.activation(
                    s0_sb, s0_ps, mybir.ActivationFunctionType.Copy, scale=0.1
                )
                _squash(nc, small, s0_sb, B, v0)  # [32, 160]

            state = {}  # blk -> dict(u_hat, blog, blog2, vcur)
            route_n = [0]  # tail emission counter (cbd buffer parity)

            def build(blk):
                u_hat = uhat_pool.tile([128, JJ, OD], F16, name="u_hat")
                state[blk] = {"u_hat": u_hat}
                for ch in range(8):  # 9 jj per chunk
                    bdu = bdu_pool.tile([128, 9, BB, G], F16, name="bdu")
                    nc.sync.dma_start(
                        out=bdu,
                        in_=bdu_d[
                            (blk * 8 + ch) * 128 : (blk * 8 + ch + 1) * 128, :
                        ].rearrange("p (a b g) -> p a b g", b=BB, g=G),
                    )

                    for j3 in range(3):  # 3-jj groups share one psum bank
                        ps = pb_pool.tile([128, 3, OD], F32, name="ps")
                        for j in range(3):
                            jj = ch * 9 + j3 * 3 + j
                            nc.tensor.matmul(
                                ps[:, j, :], lhsT=bdu[:, j3 * 3 + j, :, :],
                                rhs=wr[:, jj, :], start=True, stop=True,
                            )
                        jj0 = ch * 9 + j3 * 3
                        nc.scalar.copy(u_hat[:, jj0 : jj0 + 3, :], ps)

            def route_agree(blk, t):
                st = state[blk]
                u_hat = st["u_hat"]
                # -- vrep[(b,g), od] = v_{t-1}[b, od] --
                vrep = small.tile([128, OD], F32, tag="vrep", name="vrep")
                if t == 1:
                    src = _ap(v0[:], [[0, G], [1, OD]])
                    src = bass.AP(
                        tensor=src.tensor,
                        offset=src.offset + blk * BB * v0[:].ap[0][0],
                        ap=[[v0[:].ap[0][0], BB]] + list(src.ap)[1:],
                    )
                else:
                    vtmp = small.tile([BB, OD], F32, tag="vtmp", name="vtmp",
                                      bufs=4)
                    nc.scalar.dma_start(out=vtmp, in_=st["vcur"])
                    vt = vtmp[:]
                    src = bass.AP(
                        tensor=vt.tensor, offset=vt.offset,
                        ap=[[vt.ap[0][0], BB], [0, G], [1, OD]],
                    )
                nc.scalar.dma_start(out=vrep, in_=src)

                # -- agreement a[b,i,o] = sum_d u_hat*v into bl --
                if t == 1:
                    bl = blog_pool.tile([128, JJ, O], F16, tag="blog",
                                        name="blog", bufs=3)
                    st["blog"] = bl
                else:
                    bl = blog_pool.tile([128, JJ, O], F16, tag="blog2",
                                        name="blog2", bufs=1)
                AC = AC_LAST if (blk >= NBLK - 2 and t == 2) else AC_DEF
                PJJ = JJ - NDVE * AC
                # DVE chunks: fused scan-MAC cumsum, segment sums by
                # strided subtraction against the zero-led scratch.
                for h in range(NDVE):
                    nc.vector._custom_dve(
                        _SCAN_MAC,
                        out=scr[:, 1 : 1 + AC * OD],
                        in0=u_hat[:, h * AC : (h + 1) * AC, :],
                        in1=_ap(vrep[:], [[0, AC], [1, OD]]),
                    )
                    sv = scr[:]
                    nseg = AC * O
                    s_hi = _ap(sv, [[D, nseg]], extra_offset=D)
                    s_lo = _ap(sv, [[D, nseg]], extra_offset=0)
                    blf = bl[:, h * AC : (h + 1) * AC, :].rearrange(
                        "p a o -> p (a o)"
                    )
                    nc.vector.tensor_sub(blf, s_hi, s_lo)
                # Pool chunk: products then 4-level tree reduce over d.
                jp = NDVE * AC
                nsegp = PJJ * O
                pp = pp_pool.tile([128, PJJ * OD], F16, name="pp", bufs=2)
                nc.gpsimd.tensor_mul(
                    pp,
                    u_hat[:, jp:, :].rearrange("p a od -> p (a od)"),
                    _ap(vrep[:], [[0, PJJ], [1, OD]]),
                )
                tr1 = pp_pool.tile([128, nsegp * 8], F16, tag="tr1",
                                   name="tr1", bufs=1)
                nc.gpsimd.tensor_add(
                    tr1[:].rearrange("p (s d) -> p s d", d=8),
                    _ap(pp[:], [[D, nsegp], [1, 8]]),
                    _ap(pp[:], [[D, nsegp], [1, 8]], extra_offset=8),
                )
                tr2 = pp_pool.tile([128, nsegp * 4], F16, tag="tr2",
                                   name="tr2", bufs=1)
                nc.gpsimd.tensor_add(
                    tr2[:].rearrange("p (s d) -> p s d", d=4),
                    _ap(tr1[:], [[8, nsegp], [1, 4]]),
                    _ap(tr1[:], [[8, nsegp], [1, 4]], extra_offset=4),
                )
                tr3 = pp_pool.tile([128, nsegp * 2], F16, tag="tr3",
                                   name="tr3", bufs=1)
                nc.gpsimd.tensor_add(
                    tr3[:].rearrange("p (s d) -> p s d", d=2),
                    _ap(tr2[:], [[4, nsegp], [1, 2]]),
                    _ap(tr2[:], [[4, nsegp], [1, 2]], extra_offset=2),
                )
                blfp = bl[:, jp:, :].rearrange("p a o -> p (a o)")
                nc.gpsimd.tensor_add(
                    blfp,
                    _ap(tr3[:], [[2, nsegp]]),
                    _ap(tr3[:], [[2, nsegp]], extra_offset=1),
                )

                st["bl"] = bl

            def route_tail(blk, t):
                st = state[blk]
                u_hat = st["u_hat"]
                bl = st["bl"]
                # -- c = softmax(blog[+blog2]) over o --
                if t == 1:
                    blogs = bl
                else:
                    blogs = st["blog"]
                    nc.vector.tensor_add(blogs, blogs, bl)
                cb32 = cbuf_pool.tile([128, JJ, O], F32, tag="cb32",
                                      name="cb32")
                nc.scalar.activation(
                    cb32, blogs, mybir.ActivationFunctionType.Exp
                )
                ssum = small.tile([128, JJ], F32, tag="ssum", name="ssum")
                nc.vector.reduce_sum(
                    out=ssum, in_=cb32, axis=mybir.AxisListType.X
                )
                rec = small.tile([128, JJ], F32, tag="srec", name="srec")
                nc.vector.reciprocal(rec, ssum)
                cb2 = cbuf_pool.tile([128, JJ, O], F16, tag="cb2", name="cb2")
                nc.vector.tensor_mul(
                    cb2, cb32, _ap(rec[:], [[1, JJ], [0, O]])
                )

                # -- cbd = cb2 (bcast over b') * maskb (bcast over jj) --
                # all-f16 packed SBUF operands -> DVE 2x mode; two jj-halves
                # so the s-matmul can start on the first half earlier
                cbd = cbd_t[route_n[0] % 2]
                route_n[0] += 1
                JH = JJ // 2
                for half in range(2):
                    nc.vector.tensor_mul(
                        cbd[:, half * JH : (half + 1) * JH, :, :],
                        _ap(cb2[:], [[O, JH], [0, BB], [1, O]],
                            extra_offset=half * JH * O),
                        _ap(maskb[:], [[0, JH], [O, BB], [1, O]]),
                    )

                # -- s matmul: accumulate over jj --
                s_ps = ps_pool.tile([BB * O, OD], F32, name="s_ps")
                for jj in range(JJ):
                    nc.tensor.matmul(
                        s_ps, lhsT=cbd[:, jj, :, :], rhs=u_hat[:, jj, :],
                        start=(jj == 0), stop=(jj == JJ - 1),
                    )

                # -- diag extract: s80[(b,o), d] = s_ps[(b,o), o*16+d] --
                sdm = small.tile([O * BB, OD], F32, tag="sdm", name="sdm")
                nc.vector.tensor_mul(sdm, s_ps, maskd[: O * BB, :])
                s80 = small.tile([O * BB, D], F32, tag="s80", name="s80")
                nc.vector.reduce_sum(
                    out=s80,
                    in_=sdm[:].rearrange("p (o d) -> p d o", d=D),
                    axis=mybir.AxisListType.X,
                )
                # squash on [(b,o), d] with per-partition scalars
                nsq = small.tile([O * BB, 1], F32, tag="nsq80", name="nsq")
                sq = small.tile([O * BB, D], F32, tag="sq80", name="sq")
                nc.scalar.square(sq, s80)
                nc.vector.reduce_sum(
                    out=nsq, in_=sq, axis=mybir.AxisListType.X
                )
                rt = small.tile([O * BB, 1], F32, tag="rt80", name="rt")
                nc.scalar.activation(
                    rt, nsq, mybir.ActivationFunctionType.Ln
                )
                nc.scalar.activation(
                    rt, rt, mybir.ActivationFunctionType.Exp, scale=0.5
                )
                op1 = small.tile([O * BB, 1], F32, tag="op180", name="op1")
                nc.vector.tensor_scalar_add(op1, nsq, 1.0)
                rec80 = small.tile([O * BB, 1], F32, tag="rec80", name="rec80")
                nc.vector.reciprocal(rec80, op1)
                nc.vector.tensor_mul(rec80, rec80, rt)
                vcur = small.tile([O * BB, D], F32, tag="vcur", name="vcur",
                                  bufs=4)
                nc.vector.tensor_scalar_mul(vcur, s80, rec80)
                st["vcur"] = vcur

                if t == N_ITERS - 1:
                    # v_out[blk*8+b, o*16+d] = vcur[b*10+o, d] (same order)
                    nc.scalar.dma_start(
                        out=out_d[blk * BB : (blk + 1) * BB, :], in_=vcur
                    )

            # software pipeline: stage s emits build(s), then both routes'
            # agreement phases (DVE scans + Pool tree) before either tail
            # (softmax/cbd/s-matmul/squash), so ready scans are never stuck
            # behind a tail blocked on the other engine's agreement.
            compute_v0()
            for s in range(NBLK + 1):
                if s < NBLK:
                    build(s)
                if s < NBLK:
                    route_agree(s, 1)
                if s == NBLK:
                    t2s = [NBLK - 2, NBLK - 1]  # drain: both remaining t2s
                elif 2 <= s:
                    t2s = [s - 2]
                else:
                    t2s = []
                for n in t2s:
                    route_agree(n, 2)
                for n in t2s:
                    route_tail(n, 2)
                if s < NBLK:
                    route_tail(s, 1)
    nc.compile()
    return nc


# ---------------- host side ----------------

_NC_CACHE = None


def _get_nc():
    global _NC_CACHE
    if _NC_CACHE is None:
        _NC_CACHE = build_program()
    return _NC_CACHE


def _pack_wr(W):
    # Wr[g*8+k, jj*160 + o*16 + d] = W[jj*16+g, o, d, k]
    return np.ascontiguousarray(
        W.reshape(JJ, G, O, D, K).transpose(1, 4, 0, 2, 3).reshape(128, JJ * OD)
    ).astype(np.float16)


def _pack_ut(u_loc):
    # uT[g*8+k, jj*B + b] = u_loc[b, jj*16+g, k]
    return np.ascontiguousarray(
        u_loc.reshape(B, JJ, G, K).transpose(2, 3, 1, 0).reshape(128, JJ * B)
    ).astype(np.float16)


def _mask_b():
    # maskb[(b,g), (b',o)] = (b' == b), f16
    p = np.arange(128)
    mb = (np.arange(BB)[None, :] == (p // G)[:, None]).astype(np.float16)
    return np.ascontiguousarray(np.repeat(mb, O, axis=1))


def _mask_d():
    # maskd[(b,o) p<80, o'*16+d] = (o' == o); rows >=80 zero
    md = np.zeros((128, OD), dtype=np.float32)
    po = np.arange(O * BB) % O
    for od in range(OD):
        md[: O * BB, od] = (od // D == po).astype(np.float32)
    return md


def _pack_bdu(u_loc):
    # bdu[(blk,ch)*128 + g*8+k, (j, b, g')] = u_loc[blk*8+b, (ch*9+j)*16+g', k]
    #   nonzero only when g' == g; contiguous per (blk, ch) slice.
    u4 = u_loc.reshape(NBLK, BB, JJ // 9, 9, G, K)  # (blk, b, ch, j, g, k)
    out = np.zeros((NBLK, 8, G, K, 9, BB, G), dtype=np.float16)
    for g in range(G):
        # (blk, ch, k, j, b)
        out[:, :, g, :, :, :, g] = u4[:, :, :, :, g, :].transpose(0, 2, 4, 3, 1)
    return np.ascontiguousarray(out.reshape(NBLK * 8 * 128, 9 * BB * G))


LAST_RESULTS = None


def kernel(u, W):
    from concourse.bass_utils import run_bass_kernel_spmd

    global LAST_RESULTS
    u = np.asarray(u, dtype=np.float32)
    W = np.asarray(W, dtype=np.float32)
    nc = _get_nc()
    wr = _pack_wr(W)
    mb = _mask_b()
    md = _mask_d()
    in_maps = []
    for c in range(8):
        u_loc = u[c * B : (c + 1) * B]
        in_maps.append(
            {
                "wr": wr,
                "ut": _pack_ut(u_loc),
                "bdu": _pack_bdu(u_loc),
                "maskb": mb,
                "maskd": md,
            }
        )
    trace = bool(int(os.environ.get("KBENCH_TRACE", "0")))
    try:
        res = run_bass_kernel_spmd(
            nc, in_maps, core_ids=list(range(8)), trace=trace
        )
    except ModuleNotFoundError:
        # axon NTFF hook unavailable in this container; run without trace
        res = run_bass_kernel_spmd(nc, in_maps, core_ids=list(range(8)))
    LAST_RESULTS = res
    outs = [r["v_out"].reshape(B, O, D) for r in res.results]
    return np.concatenate(outs, axis=0).astype(np.float32)
